# revision 37
# baseline (speedup 1.0000x reference)
"""Trainium2 Bass kernel for nn_Mesh_Renderer: silhouette via scanline intervals.

Data-parallel over batch (core b renders view b). Host work is layout only
(gather vertices[faces], constant grid/basis tables, transpose the returned
image). All input-dependent math on device.

Device algorithm (per core):
  1. look_at camera basis from eye; projection folded as [w;1]^T @ [R^T; -R@eye]
     (40 K=4 f32 matmuls), perspective divide -> per-corner (xn, yn) [128, 40].
  2. Edge coefficients per (face, edge): e = A x + B y + C. For each pixel row
     y_i the face coverage in x is an interval [lo, hi]:
       t_k(i) = -(B_k y_i + C_k)/A_k ; edge k bounds from below iff
       sign(2*area)*A_k > 0. lo = max over lower-edges, -hi = max over upper
       (negated). Invisible/degenerate faces forced to a contributes-nothing
       interval via +-BIG offsets folded into the per-edge (u, v) small tiles;
       empty rows canonicalized with -hi' = min(-hi, -lo) (point interval).
     The t-planes t = u*y + v are evaluated by PE against a constant
     block-diagonal basis (tbasis), with (u, v) PE-transposed into lhsT.
  3. Raster: count(i,j) = sum_f([x_j >= lo] + [x_j <= hi]) = F + #covering.
     One DVE is_ge over [128, 8192] per 128-face tile computes both compares
     ([x | -x] vs [lo | -hi] broadcast over j); PE ones-matmuls accumulate
     over faces into PSUM cnt8 [8, 512] (sliding-onehot lhsT selects the row).
  4. silhouette = cnt >= F+1; DMA out; host transposes (j,i)->(i,j).
"""

import sys

if "/opt/trn_rl_repo" not in sys.path:
    sys.path.insert(0, "/opt/trn_rl_repo")

import ml_dtypes
import numpy as np

import concourse.bacc as bacc
import concourse.tile as tile
from concourse import mybir
from concourse.bass_utils import run_bass_kernel_spmd

F32 = mybir.dt.float32
BF16 = mybir.dt.bfloat16
I32 = mybir.dt.int32
OP = mybir.AluOpType
AF = mybir.ActivationFunctionType

B, V, NF, IMG = 8, 642, 1280, 64
NPIX = IMG * IMG          # 4096
NTILE = NF // 128         # 10 face tiles
NCOL = NF * 4             # 5120 gathered corners (a, b, c, a)
EPS = 1e-8
BIG = 1.0e30
TAN_T = float(np.tan(np.deg2rad(np.float32(15.0)).astype(np.float32)))


def _normalize3(nc, pool, v, name):
    """v [1,3] f32 -> v * rsqrt(sum v^2); margins cover the eps difference."""
    sq = pool.tile([1, 3], F32, name=f"{name}_sq")
    nc.vector.tensor_tensor(sq[:], v[:], v[:], OP.mult)
    s = pool.tile([1, 1], F32, name=f"{name}_s")
    nc.vector.tensor_reduce(s[:], sq[:], mybir.AxisListType.X, OP.add)
    n = pool.tile([1, 1], F32, name=f"{name}_n")
    nc.scalar.activation(n[:], s[:], AF.Sqrt)
    r = pool.tile([1, 1], F32, name=f"{name}_r")
    nc.vector.reciprocal(r[:], n[:])
    out = pool.tile([1, 3], F32, name=f"{name}_out")
    nc.vector.tensor_scalar(out[:], v[:], r[:], None, OP.mult)
    return out


def _cross3(nc, pool, a, b, name):
    a2 = pool.tile([1, 6], F32, name=f"{name}_a2")
    nc.vector.tensor_copy(a2[:, 0:3], a[:])
    nc.vector.tensor_copy(a2[:, 3:6], a[:])
    b2 = pool.tile([1, 6], F32, name=f"{name}_b2")
    nc.vector.tensor_copy(b2[:, 0:3], b[:])
    nc.vector.tensor_copy(b2[:, 3:6], b[:])
    m1 = pool.tile([1, 3], F32, name=f"{name}_m1")
    nc.vector.tensor_tensor(m1[:], a2[:, 1:4], b2[:, 2:5], OP.mult)
    m2 = pool.tile([1, 3], F32, name=f"{name}_m2")
    nc.vector.tensor_tensor(m2[:], a2[:, 2:5], b2[:, 1:4], OP.mult)
    out = pool.tile([1, 3], F32, name=f"{name}_out")
    nc.vector.tensor_tensor(out[:], m1[:], m2[:], OP.subtract)
    return out


def build_kernel(ctx, tc):
    nc = tc.nc
    vgt_d = nc.dram_tensor("vgt16", [16, NF], F32, kind="ExternalInput")
    eye_d = nc.dram_tensor("eye", [3], F32, kind="ExternalInput")
    xg_d = nc.dram_tensor("xgrid", [128, 2 * NPIX], BF16, kind="ExternalInput")
    tb_d = nc.dram_tensor("tbasis", [60, 1920], BF16, kind="ExternalInput")
    xb_d = nc.dram_tensor("xb65", [65, NPIX], BF16, kind="ExternalInput")
    sil_d = nc.dram_tensor("sil", [NPIX], F32, kind="ExternalOutput")

    cpool = ctx.enter_context(tc.tile_pool(name="cam", bufs=1))
    ppool = ctx.enter_context(tc.tile_pool(name="proj", bufs=1))
    gpool = ctx.enter_context(tc.tile_pool(name="grid", bufs=1))

    # ---- input DMAs ----
    eyeR = cpool.tile([1, 3], F32)
    nc.sync.dma_start(eyeR[:], eye_d.ap())
    vgt = gpool.tile([16, NF], F32)
    nc.sync.dma_start(vgt[:], vgt_d.ap())
    tb = gpool.tile([60, 1920], BF16)
    nc.sync.dma_start(tb[:], tb_d.ap())
    xb65 = gpool.tile([65, NPIX], BF16)
    nc.sync.dma_start(xb65[:], xb_d.ap())
    xx = gpool.tile([128, 2 * NPIX], BF16)
    xxv = xx[:].rearrange("p (s j i) -> p s j i", s=2, j=IMG)

    # identity for PE transposes (iotas on Pool, rest tiny)
    iop = gpool.tile([128, 1], I32)
    nc.gpsimd.iota(iop[:], pattern=[[1, 1]], base=0, channel_multiplier=1)
    iopf = gpool.tile([128, 1], F32)
    nc.vector.tensor_copy(iopf[:], iop[:])
    iof = gpool.tile([128, 128], I32)
    nc.gpsimd.iota(iof[:], pattern=[[1, 128]], base=0, channel_multiplier=0)
    ioff = gpool.tile([128, 128], F32)
    nc.vector.tensor_copy(ioff[:], iof[:])
    idm = gpool.tile([128, 128], F32)
    nc.vector.tensor_scalar(idm[:], ioff[:], iopf[:], None, OP.is_equal)

    # sliding one-hot for row-targeted PE accumulation
    oh = gpool.tile([128, 16], BF16)
    nc.gpsimd.memset(oh[:], 0.0)
    nc.gpsimd.memset(oh[:, 8:9], 2.0)
    oh1 = gpool.tile([128, 16], BF16)
    nc.gpsimd.memset(oh1[:], 0.0)
    nc.gpsimd.memset(oh1[:, 8:9], 1.0)

    # ---- camera basis (partition 0, tiny tiles) ----
    # x_ax dir = cross(up, z) = cross(up, -eye) up to positive scale, so the
    # x/y chain runs off -eye directly; z-normalize is off the critical path.
    nege = cpool.tile([1, 3], F32)
    nc.vector.tensor_scalar(nege[:], eyeR[:], -1.0, None, OP.mult)
    xr = cpool.tile([1, 3], F32)
    nc.vector.memset(xr[:], 0.0)
    nc.vector.tensor_copy(xr[:, 0:1], nege[:, 2:3])
    nc.vector.tensor_scalar(xr[:, 2:3], nege[:, 0:1], -1.0, None, OP.mult)
    x_ax = _normalize3(nc, cpool, xr, "nx")
    z_ax = _normalize3(nc, cpool, nege, "nz")
    y_ax = _cross3(nc, cpool, z_ax, x_ax, "cy")

    # rt16 = 4 diagonal copies of rt4 = [R^T; -(eye^T @ R^T)] (one per corner),
    # staged row-major on partition 0 and reshaped by a single DMA.
    # stage[0, r*12 + c]; block k: rows 4k+d' cols 3k+d hold R[d, d'] and row
    # 4k+3 holds -Reye[d].
    rtT9 = cpool.tile([1, 9], F32)   # rtT9[0, 3*d' + d] = axis_d[d']
    for d, axis in enumerate([x_ax, y_ax, z_ax]):
        nc.vector.tensor_copy(
            rtT9[:].rearrange("p (dp d) -> p dp d", d=3)[:, :, d], axis[:])
    # -Reye[d] = -sum_dp eye[dp] * R^T[dp, d] via elementwise + X-reduce
    el = cpool.tile([1, 9], F32)   # (d, dp) layout
    nc.vector.tensor_tensor(
        el[:].rearrange("p (d dp) -> p d dp", dp=3),
        rtT9[:].rearrange("p (dp d) -> p d dp", d=3),
        eyeR[:].unsqueeze(1).broadcast_to([1, 3, 3]), OP.mult)
    nreye0 = cpool.tile([1, 3], F32)
    nc.vector.tensor_reduce(nreye0[:], el[:].rearrange(
        "p (d dp) -> p d dp", dp=3), mybir.AxisListType.X, OP.add)
    nreye = cpool.tile([1, 3], F32)
    nc.vector.tensor_scalar(nreye[:], nreye0[:], -1.0, None, OP.mult)
    stage = cpool.tile([1, 192], F32)
    nc.vector.memset(stage[:], 0.0)
    rtv = rtT9[:].rearrange("p (dp d) -> p dp d", d=3)
    for k in range(4):
        base = 51 * k  # block k: coord rows at 51k + 12d' + d, ones at +36+d
        nc.vector.tensor_copy(
            stage[:, base : base + 36].rearrange(
                "p (dp c) -> p dp c", c=12)[:, :, 0:3], rtv)
        nc.vector.tensor_copy(stage[:, base + 36 : base + 39], nreye[:])
    rt16 = cpool.tile([16, 12], F32)
    nc.sync.dma_start(rt16[:], stage[:])

    # ---- projection: vca[p, (ft, k, d)] = [w;1]^T @ rt4 per corner ----
    vca = ppool.tile([128, 120], F32)
    with tc.tile_pool(name="pvc", bufs=1, space="PSUM") as psvc:
        vcp = psvc.tile([128, 120], F32)
        for ft in range(NTILE):
            nc.tensor.matmul(
                vcp[:, 12 * ft : 12 * (ft + 1)],
                vgt[:, 128 * ft : 128 * (ft + 1)],
                rt16[:],
                start=True,
                stop=True,
            )
        nc.vector.tensor_copy(vca[:], vcp[:])

    # junk write that depends on vca: pins the xgrid DMA behind the
    # projection in the scheduler so its long transfer cannot delay rt16's
    nc.vector.tensor_copy(xx[:, 0:1], vca[:, 0:1])
    nc.sync.dma_start(xx[:], xg_d.ap())
    # keep PE p-state ramped between projection and the T matmuls
    with tc.tile_pool(name="pwarm0", bufs=1, space="PSUM") as pwarm0:
        wps0 = pwarm0.tile([128, 480], F32, tag="wps0")
        for _ in range(14):
            nc.tensor.matmul(wps0[:], tb[:, 0:128], tb[:, 0:480], start=True,
                             stop=True)
    vcav = vca[:].rearrange("p (c d) -> p c d", d=3)
    vx, vy, vz = vcav[:, :, 0], vcav[:, :, 1], vcav[:, :, 2]

    # perspective divide (raw reciprocal; interval margins tolerate ~3e-3)
    dn = ppool.tile([128, 40], F32)
    nc.vector.tensor_scalar(dn[:], vz, TAN_T, EPS, OP.mult, OP.add)
    rc = ppool.tile([128, 40], F32)
    nc.vector.reciprocal(rc[:], dn[:])
    xn = ppool.tile([128, 40], F32)
    nc.vector.tensor_tensor(xn[:], vx, rc[:], OP.mult)
    yn = ppool.tile([128, 40], F32)
    nc.vector.tensor_tensor(yn[:], vy, rc[:], OP.mult)

    # visibility: all corner z > 0 (on Pool)
    vz4 = vca[:].rearrange("p (ft k d) -> p ft k d", k=4, d=3)
    mz1 = ppool.tile([128, 10], F32)
    nc.vector.tensor_tensor(mz1[:], vz4[:, :, 0, 2], vz4[:, :, 1, 2], OP.min)
    mz = ppool.tile([128, 10], F32)
    nc.vector.tensor_tensor(mz[:], mz1[:], vz4[:, :, 2, 2], OP.min)
    vg = ppool.tile([128, 10], F32)
    nc.vector.tensor_scalar(vg[:], mz[:], 0.0, None, OP.is_gt)

    # ---- edge coefficients [128, 30] in (ft, k) layout ----
    xn4 = xn[:].rearrange("p (ft k) -> p ft k", k=4)
    yn4 = yn[:].rearrange("p (ft k) -> p ft k", k=4)
    xk, xk1 = xn4[:, :, 0:3], xn4[:, :, 1:4]
    yk, yk1 = yn4[:, :, 0:3], yn4[:, :, 1:4]

    def t30(name, eng=None):
        return ppool.tile([128, 30], F32, name=name, tag=name)

    A = t30("A")
    Av = A[:].rearrange("p (ft k) -> p ft k", k=3)
    nc.vector.tensor_tensor(Av, yk, yk1, OP.subtract)
    Bc = t30("Bc")
    Bv = Bc[:].rearrange("p (ft k) -> p ft k", k=3)
    nc.vector.tensor_tensor(Bv, xk1, xk, OP.subtract)
    p1 = t30("p1")
    nc.gpsimd.tensor_tensor(p1[:].rearrange("p (ft k) -> p ft k", k=3), xk,
                            yk1, OP.mult)
    p2 = t30("p2")
    nc.gpsimd.tensor_tensor(p2[:].rearrange("p (ft k) -> p ft k", k=3), yk,
                            xk1, OP.mult)
    C = t30("C")
    nc.gpsimd.tensor_tensor(C[:], p1[:], p2[:], OP.subtract)

    Cv = C[:].rearrange("p (ft k) -> p ft k", k=3)
    S1 = ppool.tile([128, 10], F32, name="S1")
    nc.gpsimd.tensor_tensor(S1[:], Cv[:, :, 0], Cv[:, :, 1], OP.add)
    S = ppool.tile([128, 10], F32, name="S")
    nc.gpsimd.tensor_tensor(S[:], S1[:], Cv[:, :, 2], OP.add)

    # masks (Pool side-chain)
    w = t30("w")
    nc.gpsimd.tensor_tensor(w[:].rearrange("p (ft k) -> p ft k", k=3), Av,
                            S[:].unsqueeze(2).broadcast_to([128, 10, 3]),
                            OP.mult)
    mpos = t30("mpos")
    nc.vector.tensor_scalar(mpos[:], w[:], 0.0, None, OP.is_gt)
    mneg = t30("mneg")
    nc.vector.tensor_scalar(mneg[:], w[:], 0.0, None, OP.is_lt)
    offlo = t30("offlo")
    nc.vector.tensor_scalar(offlo[:], mpos[:], BIG, -BIG, OP.mult, OP.add)
    offnh = t30("offnh")
    nc.vector.tensor_scalar(offnh[:], mneg[:], BIG, -BIG, OP.mult, OP.add)
    mnegN = t30("mnegN")
    nc.vector.tensor_scalar(mnegN[:], mneg[:], -1.0, None, OP.mult)

    sne = ppool.tile([128, 10], F32, name="sne")
    nc.vector.tensor_scalar(sne[:], S[:], 0.0, None, OP.not_equal)
    visq = ppool.tile([128, 10], F32, name="visq")
    nc.gpsimd.tensor_tensor(visq[:], vg[:], sne[:], OP.mult)
    ivq = ppool.tile([128, 10], F32, name="ivq")
    nc.vector.tensor_scalar(ivq[:], visq[:], -2.0 * BIG, 2.0 * BIG, OP.mult,
                            OP.add)
    ivqN = ppool.tile([128, 10], F32, name="ivqN")
    nc.vector.tensor_scalar(ivqN[:], visq[:], 2.0 * BIG, -2.0 * BIG, OP.mult,
                            OP.add)

    # reciprocal side (DVE)
    iseq = t30("iseq")
    nc.vector.tensor_scalar(iseq[:], A[:], 0.0, None, OP.is_equal)
    Asafe = t30("Asafe")
    nc.vector.tensor_tensor(Asafe[:], A[:], iseq[:], OP.add)
    r0 = t30("r0")
    nc.vector.reciprocal(r0[:], Asafe[:])
    nr = t30("nr")
    nc.vector.tensor_scalar(nr[:], r0[:], -1.0, None, OP.mult)
    u = t30("u")
    nc.vector.tensor_tensor(u[:], Bc[:], nr[:], OP.mult)
    v = t30("v")
    nc.vector.tensor_tensor(v[:], C[:], nr[:], OP.mult)

    # (u, v) -> interleaved lhsT staging tiles [128, 60]: col 2m = u_m, 2m+1 = v_m
    uvlo = ppool.tile([128, 60], F32, name="uvlo")
    uvlov = uvlo[:].rearrange("p (m two) -> p m two", two=2)
    uvnh = ppool.tile([128, 60], F32, name="uvnh")
    uvnhv = uvnh[:].rearrange("p (m two) -> p m two", two=2)

    # lower side: ulo = u*mpos ; vlo = v*mpos - BIG*(1-mpos) + ivq
    nc.vector.tensor_tensor(uvlov[:, :, 0], u[:], mpos[:], OP.mult)
    vlo1 = t30("vlo1")
    nc.vector.tensor_tensor(vlo1[:], v[:], mpos[:], OP.mult)
    vlo2 = t30("vlo2")
    nc.vector.tensor_tensor(vlo2[:], vlo1[:], offlo[:], OP.add)
    nc.vector.tensor_tensor(
        uvlov[:, :, 1].rearrange("p (ft k) -> p ft k", k=3),
        vlo2[:].rearrange("p (ft k) -> p ft k", k=3),
        ivq[:].unsqueeze(2).broadcast_to([128, 10, 3]), OP.add)

    # negated upper side: unh = -u*mneg ; vnh = -v*mneg - BIG*(1-mneg) - ivq
    nc.vector.tensor_tensor(uvnhv[:, :, 0], u[:], mnegN[:], OP.mult)
    vnh1 = t30("vnh1")
    nc.vector.tensor_tensor(vnh1[:], v[:], mnegN[:], OP.mult)
    vnh2 = t30("vnh2")
    nc.vector.tensor_tensor(vnh2[:], vnh1[:], offnh[:], OP.add)
    nc.vector.tensor_tensor(
        uvnhv[:, :, 1].rearrange("p (ft k) -> p ft k", k=3),
        vnh2[:].rearrange("p (ft k) -> p ft k", k=3),
        ivqN[:].unsqueeze(2).broadcast_to([128, 10, 3]), OP.add)

    # ---- T planes via PE: transpose (u,v), matmul against constant basis ----
    TLOs = gpool.tile([128, 1920], BF16)
    TNHs = gpool.tile([128, 1920], BF16)
    with tc.tile_pool(name="ptr", bufs=2, space="PSUM") as ptr:
        uvloT = ptr.tile([60, 128], F32, tag="uvT")
        nc.tensor.transpose(uvloT[:], uvlo[:], idm[:])
        uvloB = gpool.tile([60, 128], BF16)
        nc.scalar.activation(uvloB[:], uvloT[:], AF.Copy)
        uvnhT = ptr.tile([60, 128], F32, tag="uvT")
        nc.tensor.transpose(uvnhT[:], uvnh[:], idm[:])
        uvnhB = gpool.tile([60, 128], BF16)
        nc.scalar.activation(uvnhB[:], uvnhT[:], AF.Copy)
    with tc.tile_pool(name="ptp", bufs=2, space="PSUM") as ptp:
        TLOp = ptp.tile([128, 1920], F32, tag="tp")
        for q in range(4):
            nc.tensor.matmul(TLOp[:, 480 * q : 480 * (q + 1)], uvloB[:],
                             tb[:, 480 * q : 480 * (q + 1)], start=True,
                             stop=True)
        nc.scalar.activation(TLOs[:], TLOp[:], AF.Copy)
        TNHp = ptp.tile([128, 1920], F32, tag="tp")
        for q in range(4):
            nc.tensor.matmul(TNHp[:, 480 * q : 480 * (q + 1)], uvnhB[:],
                             tb[:, 480 * q : 480 * (q + 1)], start=True,
                             stop=True)
        nc.vector.tensor_copy(TNHs[:], TNHp[:])

    # ---- chains -> LH [128, 1280]: cols (s, ft, i); s=0: lo, s=1: -hi ----
    TLOv = TLOs[:].rearrange("p (ft k i) -> p ft k i", k=3, i=IMG)
    TNHv = TNHs[:].rearrange("p (ft k i) -> p ft k i", k=3, i=IMG)
    LH = gpool.tile([128, 2 * 640], BF16)
    lo1 = gpool.tile([128, 640], BF16)
    nc.vector.tensor_tensor(lo1[:], TLOv[:, :, 0, :], TLOv[:, :, 1, :], OP.max)
    nc.vector.tensor_tensor(
        LH[:, 0:640].rearrange("p (ft i) -> p ft i", i=IMG),
        lo1[:].rearrange("p (ft i) -> p ft i", i=IMG), TLOv[:, :, 2, :],
        OP.max)
    nh1 = gpool.tile([128, 640], BF16)
    nc.vector.tensor_tensor(nh1[:], TNHv[:, :, 0, :], TNHv[:, :, 1, :], OP.max)
    nh2 = gpool.tile([128, 640], BF16)
    nc.vector.tensor_tensor(
        nh2[:].rearrange("p (ft i) -> p ft i", i=IMG),
        nh1[:].rearrange("p (ft i) -> p ft i", i=IMG), TNHv[:, :, 2, :],
        OP.max)
    # canonicalize empty rows: -hi' = min(-hi, -lo)
    nlo = gpool.tile([128, 640], BF16)
    nc.vector.tensor_scalar(nlo[:], LH[:, 0:640], -1.0, None, OP.mult)
    nc.vector.tensor_tensor(LH[:, 640:1280], nh2[:], nlo[:], OP.min)

    # ---- face-tile 9 goes through PE diff-planes + ACT Sign ----
    # d1 = x - lo, d2 = hi - x as K=65 matmuls vs the constant pixel basis
    # xb65 (rows 0..63 = onehot(i), row 64 = x_j). lhsT rows hold -lo / +hi
    # (PE-transposed from LH) with the x-coefficient in row 64.
    ACT_FT = NTILE - 1
    loP = gpool.tile([128, 65], BF16)
    nc.vector.tensor_copy(loP[:, 0:64], LH[:, 640 - 64 : 640])
    nc.vector.memset(loP[:, 64:65], -1.0)
    hiP = gpool.tile([128, 65], BF16)
    nc.vector.tensor_copy(hiP[:, 0:64], LH[:, 1280 - 64 : 1280])
    nc.vector.memset(hiP[:, 64:65], 1.0)
    idmb = gpool.tile([128, 128], BF16)
    nc.vector.tensor_copy(idmb[:], idm[:])
    lhsT1 = gpool.tile([65, 128], BF16)
    lhsT2 = gpool.tile([65, 128], BF16)
    with tc.tile_pool(name="ptd", bufs=2, space="PSUM") as ptd:
        loT = ptd.tile([65, 128], BF16, tag="dT")
        nc.tensor.transpose(loT[:], loP[:], idmb[:])
        nc.scalar.activation(lhsT1[:], loT[:], AF.Copy, scale=-1.0)
        hiT = ptd.tile([65, 128], BF16, tag="dT")
        nc.tensor.transpose(hiT[:], hiP[:], idmb[:])
        nc.scalar.activation(lhsT2[:], hiT[:], AF.Copy, scale=-1.0)
    sgn = gpool.tile([128, 2 * NPIX], BF16)

    # ---- raster: per face-tile one combined is_ge + 16 accum matmuls ----
    # Junk "warmer" matmuls keep the PE p-state ramped: a pre-raster burst
    # while the first compare runs, plus a couple per face-tile to bridge the
    # compare/accumulate rate gap without the engine ever going idle.
    LHv = LH[:].rearrange("p (s ft i) -> p s ft i", s=2, ft=NTILE)
    spool = ctx.enter_context(tc.tile_pool(name="ghp", bufs=3))
    pscnt = ctx.enter_context(tc.tile_pool(name="pcnt", bufs=1, space="PSUM"))
    pwarm = ctx.enter_context(tc.tile_pool(name="pwarm", bufs=1, space="PSUM"))
    cnt8 = pscnt.tile([8, 512], F32, tag="cnt8")
    wps = pwarm.tile([128, 480], F32, tag="wps")

    def warm(n):
        for wq in range(n):
            nc.tensor.matmul(wps[:], uvloB[:], tb[:, 0:480], start=True,
                             stop=True)

    warm(10)
    pdif = ctx.enter_context(tc.tile_pool(name="pdif", bufs=1, space="PSUM"))
    nmm = 0
    NMM = (NTILE - 1) * 16

    def diff_half(h):
        side, hh = h // 2, h % 2
        lhsT = lhsT1 if side == 0 else lhsT2
        dp = pdif.tile([128, 2048], F32, tag="dp")
        for q in range(4):
            off = 2048 * hh + 512 * q
            nc.tensor.matmul(dp[:, 512 * q : 512 * (q + 1)], lhsT[:],
                             xb65[:, off : off + 512], start=True, stop=True)
        return dp

    def sign_half(h, dp):
        nc.scalar.activation(sgn[:, 2048 * h : 2048 * (h + 1)], dp[:], AF.Sign)

    def sgn_accum(slot):
        for c in range(4 * slot, 4 * slot + 4):
            q = c % 8
            nc.tensor.matmul(cnt8[:, :], oh1[:, 8 - q : 16 - q],
                             sgn[:, 512 * c : 512 * (c + 1)],
                             start=False, stop=False)

    def accum16(part, crange, base):
        nonlocal nmm
        for c in crange:
            q = c % 8
            nc.tensor.matmul(cnt8[:, :], oh[:, 8 - q : 16 - q],
                             part[:, 512 * (c - base) : 512 * (c - base + 1)],
                             start=(nmm == 0), stop=(nmm == NMM - 1))
            nmm += 1

    dps = {0: diff_half(0)}
    for ft in range(NTILE - 1):
        ghp = spool.tile([128, 2 * NPIX], BF16, tag="ghp")
        lhb = LHv[:, :, ft, :].unsqueeze(2).broadcast_to([128, 2, IMG, IMG])
        nc.vector.tensor_tensor(
            ghp[:].rearrange("p (s j i) -> p s j i", s=2, j=IMG), xxv, lhb,
            OP.is_ge)
        accum16(ghp, range(16), 0)
        if ft <= 3:
            sign_half(ft, dps.pop(ft))
            if ft < 3:
                dps[ft + 1] = diff_half(ft + 1)
        elif ft <= 7:
            sgn_accum(ft - 4)

    # ---- threshold: covered iff cnt >= NF + 1 ----
    silb = gpool.tile([8, 512], F32)
    nc.vector.tensor_scalar(silb[:], cnt8[:], 2.0 * (NF - 128) + 0.5, None,
                            OP.is_gt)
    nc.sync.dma_start(sil_d.ap(), silb[:])


_NC = None


def _get_program():
    global _NC
    if _NC is None:
        nc = bacc.Bacc(
            "TRN2",
            target_bir_lowering=False,
            debug=False,
            enable_asserts=False,
            num_devices=B,
        )
        from contextlib import ExitStack

        with tile.TileContext(nc) as tc:
            with ExitStack() as ctx:
                build_kernel(ctx, tc)
        nc.compile()
        _NC = nc
    return _NC


def _consts():
    """Input-independent constant tables (pixel grid, t-plane basis)."""
    j = np.arange(IMG, dtype=np.float32)
    xs = (2.0 * j - 63.0) / 64.0                      # exact in bf16
    ys = (63.0 - 2.0 * j) / 64.0
    xg = np.empty((2, IMG, IMG), dtype=np.float32)
    xg[0] = xs[:, None]
    xg[1] = -xs[:, None]
    xgrid = np.broadcast_to(xg.reshape(1, 2 * NPIX), (128, 2 * NPIX))
    xgrid = np.ascontiguousarray(xgrid).astype(ml_dtypes.bfloat16)
    tb = np.zeros((60, 1920), dtype=np.float32)
    for m in range(30):
        tb[2 * m, m * 64 : (m + 1) * 64] = ys
        tb[2 * m + 1, m * 64 : (m + 1) * 64] = 1.0
    tbasis = tb.astype(ml_dtypes.bfloat16)
    xb = np.zeros((65, NPIX), dtype=np.float32)
    for i in range(IMG):
        xb[i, i::IMG] = 1.0                    # onehot(i) over (j, i) columns
    xb[64] = np.repeat(xs, IMG)                # x_j
    xb65 = xb.astype(ml_dtypes.bfloat16)
    return xgrid, tbasis, xb65


def _host_layout(vertices, faces):
    """Pure indexing: gather per-face-corner vertices into [16, 1280] where
    row 4k+d / column ft*128+p holds coord d (d=3: 1.0) of corner k of face
    ft*128+p; corners are (a, b, c, a)."""
    faces4 = np.concatenate([faces, faces[:, :1]], axis=1)  # [1280, 4]
    out = []
    for b in range(B):
        vg = vertices[b][faces4]                      # [1280, 4, 3]
        vg4 = np.concatenate(
            [vg, np.ones((NF, 4, 1), dtype=np.float32)], axis=2)  # [1280,4,4]
        out.append(np.ascontiguousarray(
            vg4.transpose(1, 2, 0).reshape(16, NF).astype(np.float32)))
    return out


def kernel(vertices, viewpoints, faces, img_size):
    vertices = np.asarray(vertices, dtype=np.float32)
    viewpoints = np.asarray(viewpoints, dtype=np.float32)
    faces = np.asarray(faces, dtype=np.int32)
    assert int(img_size) == IMG and vertices.shape == (B, V, 3)

    nc = _get_program()
    vgts = _host_layout(vertices, faces)
    xgrid, tbasis, xb65 = _consts()
    in_maps = [
        {"vgt16": vgts[b], "eye": np.ascontiguousarray(viewpoints[b]),
         "xgrid": xgrid, "tbasis": tbasis, "xb65": xb65}
        for b in range(B)
    ]
    res = run_bass_kernel_spmd(nc, in_maps, core_ids=list(range(B)))
    # device pixel order is (j, i): transpose back to raster (i, j)
    sil = np.stack([
        res.results[b]["sil"].reshape(IMG, IMG).T for b in range(B)
    ])
    return sil.reshape(B, 1, IMG, IMG).astype(np.float32)


if __name__ == "__main__":
    rng = np.random.default_rng(0)
    verts = rng.standard_normal((B, V, 3), dtype=np.float32) * 0.5
    vps = rng.standard_normal((B, 3), dtype=np.float32)
    fcs = rng.integers(0, V, (NF, 3), dtype=np.int32)
    out = kernel(verts, vps, fcs, IMG)
    print(out.shape, out.sum())


# revision 51
# speedup vs baseline: 1.0563x; 1.0563x over previous
"""Trainium2 Bass kernel for nn_Mesh_Renderer: silhouette via scanline intervals.

Data-parallel over batch (core b renders view b). Host work is layout only
(gather vertices[faces], constant grid/basis tables, transpose the returned
image). All input-dependent math on device.

Device algorithm (per core):
  1. look_at camera basis from eye; projection folded as [w;1]^T @ [R^T; -R@eye]
     (40 K=4 f32 matmuls), perspective divide -> per-corner (xn, yn) [128, 40].
  2. Edge coefficients per (face, edge): e = A x + B y + C. For each pixel row
     y_i the face coverage in x is an interval [lo, hi]:
       t_k(i) = -(B_k y_i + C_k)/A_k ; edge k bounds from below iff
       sign(2*area)*A_k > 0. lo = max over lower-edges, -hi = max over upper
       (negated). Invisible/degenerate faces forced to a contributes-nothing
       interval via +-BIG offsets folded into the per-edge (u, v) small tiles;
       empty rows canonicalized with -hi' = min(-hi, -lo) (point interval).
     The t-planes t = u*y + v are evaluated by PE against a constant
     block-diagonal basis (tbasis), with (u, v) PE-transposed into lhsT.
  3. Raster: count(i,j) = sum_f([x_j >= lo] + [x_j <= hi]) = F + #covering.
     One DVE is_ge over [128, 8192] per 128-face tile computes both compares
     ([x | -x] vs [lo | -hi] broadcast over j); PE ones-matmuls accumulate
     over faces into PSUM cnt8 [8, 512] (sliding-onehot lhsT selects the row).
  4. silhouette = cnt >= F+1; DMA out; host transposes (j,i)->(i,j).
"""

import sys

if "/opt/trn_rl_repo" not in sys.path:
    sys.path.insert(0, "/opt/trn_rl_repo")

import ml_dtypes
import numpy as np

import concourse.bacc as bacc
import concourse.tile as tile
from concourse import mybir
from concourse.bass_utils import run_bass_kernel_spmd

F32 = mybir.dt.float32
BF16 = mybir.dt.bfloat16
I32 = mybir.dt.int32
OP = mybir.AluOpType
AF = mybir.ActivationFunctionType

B, V, NF, IMG = 8, 642, 1280, 64
NPIX = IMG * IMG          # 4096
NTILE = NF // 128         # 10 face tiles
NCOL = NF * 4             # 5120 gathered corners (a, b, c, a)
EPS = 1e-8
BIG = 1.0e30
TAN_T = float(np.tan(np.deg2rad(np.float32(15.0)).astype(np.float32)))


def _normalize3(nc, pool, v, name):
    """v [1,3] f32 -> v * rsqrt(sum v^2); margins cover the eps difference."""
    sq = pool.tile([1, 3], F32, name=f"{name}_sq")
    nc.vector.tensor_tensor(sq[:], v[:], v[:], OP.mult)
    s = pool.tile([1, 1], F32, name=f"{name}_s")
    nc.vector.tensor_reduce(s[:], sq[:], mybir.AxisListType.X, OP.add)
    n = pool.tile([1, 1], F32, name=f"{name}_n")
    nc.scalar.activation(n[:], s[:], AF.Sqrt)
    r = pool.tile([1, 1], F32, name=f"{name}_r")
    nc.vector.reciprocal(r[:], n[:])
    out = pool.tile([1, 3], F32, name=f"{name}_out")
    nc.vector.tensor_scalar(out[:], v[:], r[:], None, OP.mult)
    return out


def _cross3(nc, pool, a, b, name):
    a2 = pool.tile([1, 6], F32, name=f"{name}_a2")
    nc.vector.tensor_copy(a2[:, 0:3], a[:])
    nc.vector.tensor_copy(a2[:, 3:6], a[:])
    b2 = pool.tile([1, 6], F32, name=f"{name}_b2")
    nc.vector.tensor_copy(b2[:, 0:3], b[:])
    nc.vector.tensor_copy(b2[:, 3:6], b[:])
    m1 = pool.tile([1, 3], F32, name=f"{name}_m1")
    nc.vector.tensor_tensor(m1[:], a2[:, 1:4], b2[:, 2:5], OP.mult)
    m2 = pool.tile([1, 3], F32, name=f"{name}_m2")
    nc.vector.tensor_tensor(m2[:], a2[:, 2:5], b2[:, 1:4], OP.mult)
    out = pool.tile([1, 3], F32, name=f"{name}_out")
    nc.vector.tensor_tensor(out[:], m1[:], m2[:], OP.subtract)
    return out


def build_kernel(ctx, tc):
    nc = tc.nc
    vgt_d = nc.dram_tensor("vgt16", [16, NF], F32, kind="ExternalInput")
    eye_d = nc.dram_tensor("eye", [3], F32, kind="ExternalInput")
    xg_d = nc.dram_tensor("xgrid", [128, 2 * NPIX], BF16, kind="ExternalInput")
    tb_d = nc.dram_tensor("tbasis", [60, 1920], BF16, kind="ExternalInput")
    xb_d = nc.dram_tensor("xb65", [65, NPIX], BF16, kind="ExternalInput")
    sil_d = nc.dram_tensor("sil", [NPIX], F32, kind="ExternalOutput")

    cpool = ctx.enter_context(tc.tile_pool(name="cam", bufs=1))
    ppool = ctx.enter_context(tc.tile_pool(name="proj", bufs=1))
    gpool = ctx.enter_context(tc.tile_pool(name="grid", bufs=1))

    # ---- input DMAs ----
    eyeR = cpool.tile([1, 3], F32)
    nc.sync.dma_start(eyeR[:], eye_d.ap())
    vgt = gpool.tile([16, NF], F32)
    nc.sync.dma_start(vgt[:], vgt_d.ap())
    tb = gpool.tile([60, 1920], BF16)
    nc.sync.dma_start(tb[:], tb_d.ap())
    xb65 = gpool.tile([65, NPIX], BF16)
    nc.sync.dma_start(xb65[:], xb_d.ap())
    xx = gpool.tile([128, 2 * NPIX], BF16)
    xxv = xx[:].rearrange("p (s j i) -> p s j i", s=2, j=IMG)

    # identity for PE transposes (iotas on Pool, rest tiny)
    iop = gpool.tile([128, 1], I32)
    nc.gpsimd.iota(iop[:], pattern=[[1, 1]], base=0, channel_multiplier=1)
    iopf = gpool.tile([128, 1], F32)
    nc.vector.tensor_copy(iopf[:], iop[:])
    iof = gpool.tile([128, 128], I32)
    nc.gpsimd.iota(iof[:], pattern=[[1, 128]], base=0, channel_multiplier=0)
    ioff = gpool.tile([128, 128], F32)
    nc.vector.tensor_copy(ioff[:], iof[:])
    idm = gpool.tile([128, 128], F32)
    nc.vector.tensor_scalar(idm[:], ioff[:], iopf[:], None, OP.is_equal)

    # pixel-row y values for the DVE-side TLO build
    it32 = gpool.tile([128, IMG], I32)
    nc.gpsimd.iota(it32[:], pattern=[[1, IMG]], base=0, channel_multiplier=0)
    itf = gpool.tile([128, IMG], F32)
    nc.vector.tensor_copy(itf[:], it32[:])
    ysb = gpool.tile([128, IMG], BF16)   # y_i = (63 - 2i)/64, exact bf16
    nc.vector.tensor_scalar(ysb[:], itf[:], -1.0 / 32.0, 63.0 / 64.0, OP.mult,
                            OP.add)

    # sliding one-hot for row-targeted PE accumulation
    oh = gpool.tile([128, 16], BF16)
    nc.gpsimd.memset(oh[:], 0.0)
    nc.gpsimd.memset(oh[:, 8:9], 2.0)
    oh1 = gpool.tile([128, 16], BF16)
    nc.gpsimd.memset(oh1[:], 0.0)
    nc.gpsimd.memset(oh1[:, 8:9], 1.0)

    # ---- camera basis (partition 0, tiny tiles) ----
    # x_ax dir = cross(up, z) = cross(up, -eye) up to positive scale; the
    # x-chain (DVE) and z-chain (Pool/ACT) run in parallel. Normalized axes
    # are produced directly in duplicated [1, 6] form for the cross product.
    nege = cpool.tile([1, 3], F32)
    nc.vector.tensor_scalar(nege[:], eyeR[:], -1.0, None, OP.mult)
    xr = cpool.tile([1, 3], F32)
    nc.vector.memset(xr[:], 0.0)
    nc.vector.tensor_copy(xr[:, 0:1], nege[:, 2:3])
    nc.vector.tensor_scalar(xr[:, 2:3], nege[:, 0:1], -1.0, None, OP.mult)
    # x chain (DVE)
    sqx = cpool.tile([1, 3], F32)
    nc.vector.tensor_tensor(sqx[:], xr[:], xr[:], OP.mult)
    ssx = cpool.tile([1, 1], F32)
    nc.vector.tensor_reduce(ssx[:], sqx[:], mybir.AxisListType.X, OP.add)
    nx_ = cpool.tile([1, 1], F32)
    nc.scalar.activation(nx_[:], ssx[:], AF.Sqrt)
    rx_ = cpool.tile([1, 1], F32)
    nc.vector.reciprocal(rx_[:], nx_[:])
    xdup = cpool.tile([1, 6], F32)
    nc.vector.tensor_scalar(
        xdup[:].rearrange("p (two d) -> p two d", d=3),
        xr[:].unsqueeze(1).broadcast_to([1, 2, 3]), rx_[:], None, OP.mult)
    # z chain (Pool + ACT, parallel to the x chain)
    sqz = cpool.tile([1, 3], F32)
    nc.gpsimd.tensor_tensor(sqz[:], nege[:], nege[:], OP.mult)
    ssz1 = cpool.tile([1, 1], F32)
    nc.gpsimd.tensor_tensor(ssz1[:], sqz[:, 0:1], sqz[:, 1:2], OP.add)
    ssz = cpool.tile([1, 1], F32)
    nc.gpsimd.tensor_tensor(ssz[:], ssz1[:], sqz[:, 2:3], OP.add)
    nz_ = cpool.tile([1, 1], F32)
    nc.scalar.activation(nz_[:], ssz[:], AF.Sqrt)
    rz_ = cpool.tile([1, 1], F32)
    nc.vector.reciprocal(rz_[:], nz_[:])
    zdup = cpool.tile([1, 6], F32)
    nc.vector.tensor_scalar(
        zdup[:].rearrange("p (two d) -> p two d", d=3),
        nege[:].unsqueeze(1).broadcast_to([1, 2, 3]), rz_[:], None, OP.mult)
    # y = cross(z, x) of unit orthogonal vectors (unit up to ~1e-7)
    m1 = cpool.tile([1, 3], F32)
    nc.vector.tensor_tensor(m1[:], zdup[:, 1:4], xdup[:, 2:5], OP.mult)
    m2 = cpool.tile([1, 3], F32)
    nc.vector.tensor_tensor(m2[:], zdup[:, 2:5], xdup[:, 1:4], OP.mult)
    y_ax = cpool.tile([1, 3], F32)
    nc.vector.tensor_tensor(y_ax[:], m1[:], m2[:], OP.subtract)
    x_ax, z_ax = xdup, zdup

    # rt16 = 4 diagonal copies of rt4 = [R^T; -(eye^T @ R^T)] (one per corner),
    # staged row-major on partition 0 and reshaped by a single DMA.
    # stage[0, r*12 + c]; block k: rows 4k+d' cols 3k+d hold R[d, d'] and row
    # 4k+3 holds -Reye[d].
    rtT9 = cpool.tile([1, 9], F32)   # rtT9[0, 3*d' + d] = axis_d[d']
    for d, axis in enumerate([x_ax, y_ax, z_ax]):
        nc.vector.tensor_copy(
            rtT9[:].rearrange("p (dp d) -> p dp d", d=3)[:, :, d],
            axis[:, 0:3])
    # -Reye[d] = -sum_dp eye[dp] * R^T[dp, d] via elementwise + X-reduce
    el = cpool.tile([1, 9], F32)   # (d, dp) layout
    nc.vector.tensor_tensor(
        el[:].rearrange("p (d dp) -> p d dp", dp=3),
        rtT9[:].rearrange("p (dp d) -> p d dp", d=3),
        eyeR[:].unsqueeze(1).broadcast_to([1, 3, 3]), OP.mult)
    nreye0 = cpool.tile([1, 3], F32)
    nc.vector.tensor_reduce(nreye0[:], el[:].rearrange(
        "p (d dp) -> p d dp", dp=3), mybir.AxisListType.X, OP.add)
    nreye = cpool.tile([1, 3], F32)
    nc.vector.tensor_scalar(nreye[:], nreye0[:], -1.0, None, OP.mult)
    stage = cpool.tile([1, 192], F32)
    nc.vector.memset(stage[:], 0.0)
    rtv = rtT9[:].rearrange("p (dp d) -> p dp d", d=3)
    for k in range(4):
        base = 51 * k  # block k: coord rows at 51k + 12d' + d, ones at +36+d
        nc.vector.tensor_copy(
            stage[:, base : base + 36].rearrange(
                "p (dp c) -> p dp c", c=12)[:, :, 0:3], rtv)
        nc.vector.tensor_copy(stage[:, base + 36 : base + 39], nreye[:])
    rt16 = cpool.tile([16, 12], F32)
    nc.sync.dma_start(rt16[:], stage[:])

    # ---- projection: vca[p, (ft, k, d)] = [w;1]^T @ rt4 per corner ----
    vca = ppool.tile([128, 120], F32)
    with tc.tile_pool(name="pvc", bufs=1, space="PSUM") as psvc:
        vcp = psvc.tile([128, 120], F32)
        for ft in range(NTILE):
            nc.tensor.matmul(
                vcp[:, 12 * ft : 12 * (ft + 1)],
                vgt[:, 128 * ft : 128 * (ft + 1)],
                rt16[:],
                start=True,
                stop=True,
            )
        nc.vector.tensor_copy(vca[:], vcp[:])

    # junk write that depends on vca: pins the xgrid DMA behind the
    # projection in the scheduler so its long transfer cannot delay rt16's
    nc.vector.tensor_copy(xx[:, 0:1], vca[:, 0:1])
    nc.sync.dma_start(xx[:], xg_d.ap())
    # keep PE p-state ramped between projection and the T matmuls
    with tc.tile_pool(name="pwarm0", bufs=1, space="PSUM") as pwarm0:
        wps0 = pwarm0.tile([128, 480], F32, tag="wps0")
        for _ in range(14):
            nc.tensor.matmul(wps0[:], tb[:, 0:128], tb[:, 0:480], start=True,
                             stop=True)
    vcav = vca[:].rearrange("p (c d) -> p c d", d=3)
    vx, vy, vz = vcav[:, :, 0], vcav[:, :, 1], vcav[:, :, 2]

    # perspective divide (raw reciprocal; interval margins tolerate ~3e-3)
    dn = ppool.tile([128, 40], F32)
    nc.vector.tensor_scalar(dn[:], vz, TAN_T, EPS, OP.mult, OP.add)
    rc = ppool.tile([128, 40], F32)
    nc.vector.reciprocal(rc[:], dn[:])
    xn = ppool.tile([128, 40], F32)
    nc.vector.tensor_tensor(xn[:], vx, rc[:], OP.mult)
    yn = ppool.tile([128, 40], F32)
    nc.vector.tensor_tensor(yn[:], vy, rc[:], OP.mult)

    # visibility: all corner z > 0 (on Pool)
    vz4 = vca[:].rearrange("p (ft k d) -> p ft k d", k=4, d=3)
    mz1 = ppool.tile([128, 10], F32)
    nc.vector.tensor_tensor(mz1[:], vz4[:, :, 0, 2], vz4[:, :, 1, 2], OP.min)
    mz = ppool.tile([128, 10], F32)
    nc.vector.tensor_tensor(mz[:], mz1[:], vz4[:, :, 2, 2], OP.min)
    vg = ppool.tile([128, 10], F32)
    nc.vector.tensor_scalar(vg[:], mz[:], 0.0, None, OP.is_gt)

    # ---- edge coefficients [128, 30] in (ft, k) layout ----
    xn4 = xn[:].rearrange("p (ft k) -> p ft k", k=4)
    yn4 = yn[:].rearrange("p (ft k) -> p ft k", k=4)
    xk, xk1 = xn4[:, :, 0:3], xn4[:, :, 1:4]
    yk, yk1 = yn4[:, :, 0:3], yn4[:, :, 1:4]

    def t30(name, eng=None):
        return ppool.tile([128, 30], F32, name=name, tag=name)

    A = t30("A")
    Av = A[:].rearrange("p (ft k) -> p ft k", k=3)
    nc.vector.tensor_tensor(Av, yk, yk1, OP.subtract)
    Bc = t30("Bc")
    Bv = Bc[:].rearrange("p (ft k) -> p ft k", k=3)
    nc.vector.tensor_tensor(Bv, xk1, xk, OP.subtract)
    p1 = t30("p1")
    nc.gpsimd.tensor_tensor(p1[:].rearrange("p (ft k) -> p ft k", k=3), xk,
                            yk1, OP.mult)
    p2 = t30("p2")
    nc.gpsimd.tensor_tensor(p2[:].rearrange("p (ft k) -> p ft k", k=3), yk,
                            xk1, OP.mult)
    C = t30("C")
    nc.gpsimd.tensor_tensor(C[:], p1[:], p2[:], OP.subtract)

    Cv = C[:].rearrange("p (ft k) -> p ft k", k=3)
    S1 = ppool.tile([128, 10], F32, name="S1")
    nc.gpsimd.tensor_tensor(S1[:], Cv[:, :, 0], Cv[:, :, 1], OP.add)
    S = ppool.tile([128, 10], F32, name="S")
    nc.gpsimd.tensor_tensor(S[:], S1[:], Cv[:, :, 2], OP.add)

    # masks (Pool side-chain)
    w = t30("w")
    nc.gpsimd.tensor_tensor(w[:].rearrange("p (ft k) -> p ft k", k=3), Av,
                            S[:].unsqueeze(2).broadcast_to([128, 10, 3]),
                            OP.mult)
    # reciprocal side (DVE)
    iseq = t30("iseq")
    nc.vector.tensor_scalar(iseq[:], A[:], 0.0, None, OP.is_equal)
    Asafe = t30("Asafe")
    nc.vector.tensor_tensor(Asafe[:], A[:], iseq[:], OP.add)
    r0 = t30("r0")
    nc.vector.reciprocal(r0[:], Asafe[:])
    nr = t30("nr")
    nc.vector.tensor_scalar(nr[:], r0[:], -1.0, None, OP.mult)
    u = t30("u")
    nc.vector.tensor_tensor(u[:], Bc[:], nr[:], OP.mult)
    v = t30("v")
    nc.vector.tensor_tensor(v[:], C[:], nr[:], OP.mult)
    mpos = t30("mpos")
    nc.vector.tensor_scalar(mpos[:], w[:], 0.0, None, OP.is_gt)
    mneg = t30("mneg")
    nc.vector.tensor_scalar(mneg[:], w[:], 0.0, None, OP.is_lt)
    offlo = t30("offlo")
    nc.vector.tensor_scalar(offlo[:], mpos[:], BIG, -BIG, OP.mult, OP.add)
    offnh = t30("offnh")
    nc.vector.tensor_scalar(offnh[:], mneg[:], BIG, -BIG, OP.mult, OP.add)
    mnegN = t30("mnegN")
    nc.vector.tensor_scalar(mnegN[:], mneg[:], -1.0, None, OP.mult)

    sne = ppool.tile([128, 10], F32, name="sne")
    nc.vector.tensor_scalar(sne[:], S[:], 0.0, None, OP.not_equal)
    visq = ppool.tile([128, 10], F32, name="visq")
    nc.gpsimd.tensor_tensor(visq[:], vg[:], sne[:], OP.mult)
    ivq = ppool.tile([128, 10], F32, name="ivq")
    nc.vector.tensor_scalar(ivq[:], visq[:], -2.0 * BIG, 2.0 * BIG, OP.mult,
                            OP.add)
    ivqN = ppool.tile([128, 10], F32, name="ivqN")
    nc.vector.tensor_scalar(ivqN[:], visq[:], 2.0 * BIG, -2.0 * BIG, OP.mult,
                            OP.add)


    # (u, v) lhsT staging: only the negated-hi side goes through PE
    uvnh = ppool.tile([128, 60], F32, name="uvnh")
    uvnhv = uvnh[:].rearrange("p (m two) -> p m two", two=2)

    # lower side (computed on DVE directly): ulo = u*mpos ;
    # vlo = v*mpos - BIG*(1-mpos) + ivq
    ulo = t30("ulo")
    nc.vector.tensor_tensor(ulo[:], u[:], mpos[:], OP.mult)
    vlo1 = t30("vlo1")
    nc.vector.tensor_tensor(vlo1[:], v[:], mpos[:], OP.mult)
    vlo2 = t30("vlo2")
    nc.vector.tensor_tensor(vlo2[:], vlo1[:], offlo[:], OP.add)
    vlo = t30("vlo")
    nc.vector.tensor_tensor(
        vlo[:].rearrange("p (ft k) -> p ft k", k=3),
        vlo2[:].rearrange("p (ft k) -> p ft k", k=3),
        ivq[:].unsqueeze(2).broadcast_to([128, 10, 3]), OP.add)
    ulob = ppool.tile([128, 30], BF16, name="ulob")
    nc.vector.tensor_copy(ulob[:], ulo[:])
    vlob = ppool.tile([128, 30], BF16, name="vlob")
    nc.vector.tensor_copy(vlob[:], vlo[:])

    # negated upper side: unh = -u*mneg ; vnh = -v*mneg - BIG*(1-mneg) - ivq
    nc.vector.tensor_tensor(uvnhv[:, :, 0], u[:], mnegN[:], OP.mult)
    vnh1 = t30("vnh1")
    nc.vector.tensor_tensor(vnh1[:], v[:], mnegN[:], OP.mult)
    vnh2 = t30("vnh2")
    nc.vector.tensor_tensor(vnh2[:], vnh1[:], offnh[:], OP.add)
    nc.vector.tensor_tensor(
        uvnhv[:, :, 1].rearrange("p (ft k) -> p ft k", k=3),
        vnh2[:].rearrange("p (ft k) -> p ft k", k=3),
        ivqN[:].unsqueeze(2).broadcast_to([128, 10, 3]), OP.add)

    # ---- T planes: TLO on DVE (bf16 broadcast ops) in parallel with the
    # negated-hi side through PE (transpose + basis matmul) + ACT drain ----
    TLOs = gpool.tile([128, 1920], BF16)
    TLOa = gpool.tile([128, 1920], BF16)
    TNHs = gpool.tile([128, 1920], BF16)
    ub3 = ulob[:].rearrange("p (ft k) -> p ft k", k=3)
    vb3 = vlob[:].rearrange("p (ft k) -> p ft k", k=3)
    ysbc = ysb[:].rearrange("p (a b i) -> p a b i", a=1, b=1).broadcast_to(
        [128, NTILE, 3, IMG])
    TLOav = TLOa[:].rearrange("p (ft k i) -> p ft k i", k=3, i=IMG)
    nc.vector.tensor_tensor(TLOav, ub3.unsqueeze(3).broadcast_to(
        [128, NTILE, 3, IMG]), ysbc, OP.mult)
    nc.vector.tensor_tensor(
        TLOs[:].rearrange("p (ft k i) -> p ft k i", k=3, i=IMG), TLOav,
        vb3.unsqueeze(3).broadcast_to([128, NTILE, 3, IMG]), OP.add)
    with tc.tile_pool(name="ptr", bufs=1, space="PSUM") as ptr:
        uvnhT = ptr.tile([60, 128], F32, tag="uvT")
        nc.tensor.transpose(uvnhT[:], uvnh[:], idm[:])
        uvnhB = gpool.tile([60, 128], BF16)
        nc.scalar.activation(uvnhB[:], uvnhT[:], AF.Copy)
    with tc.tile_pool(name="ptp", bufs=1, space="PSUM") as ptp:
        TNHp = ptp.tile([128, 1920], F32, tag="tp")
        for q in range(4):
            nc.tensor.matmul(TNHp[:, 480 * q : 480 * (q + 1)], uvnhB[:],
                             tb[:, 480 * q : 480 * (q + 1)], start=True,
                             stop=True)
        nc.scalar.activation(TNHs[:], TNHp[:], AF.Copy)

    # ---- chains -> LH [128, 1280]: cols (s, ft, i); s=0: lo, s=1: -hi ----
    TLOv = TLOs[:].rearrange("p (ft k i) -> p ft k i", k=3, i=IMG)
    TNHv = TNHs[:].rearrange("p (ft k i) -> p ft k i", k=3, i=IMG)
    LH = gpool.tile([128, 2 * 640], BF16)
    lo1 = gpool.tile([128, 640], BF16)
    nc.vector.tensor_tensor(lo1[:], TLOv[:, :, 0, :], TLOv[:, :, 1, :], OP.max)
    nc.vector.tensor_tensor(
        LH[:, 0:640].rearrange("p (ft i) -> p ft i", i=IMG),
        lo1[:].rearrange("p (ft i) -> p ft i", i=IMG), TLOv[:, :, 2, :],
        OP.max)
    nh1 = gpool.tile([128, 640], BF16)
    nc.vector.tensor_tensor(nh1[:], TNHv[:, :, 0, :], TNHv[:, :, 1, :], OP.max)
    nh2 = gpool.tile([128, 640], BF16)
    nc.vector.tensor_tensor(
        nh2[:].rearrange("p (ft i) -> p ft i", i=IMG),
        nh1[:].rearrange("p (ft i) -> p ft i", i=IMG), TNHv[:, :, 2, :],
        OP.max)
    # canonicalize empty rows: -hi' = min(-hi, -lo)
    nlo = gpool.tile([128, 640], BF16)
    nc.vector.tensor_scalar(nlo[:], LH[:, 0:640], -1.0, None, OP.mult)
    nc.vector.tensor_tensor(LH[:, 640:1280], nh2[:], nlo[:], OP.min)

    # ---- face-tile 9 goes through PE diff-planes + ACT Sign ----
    # d1 = x - lo, d2 = hi - x as K=65 matmuls vs the constant pixel basis
    # xb65 (rows 0..63 = onehot(i), row 64 = x_j). lhsT rows hold -lo / +hi
    # (PE-transposed from LH) with the x-coefficient in row 64.
    ACT_FT = NTILE - 1
    loP = gpool.tile([128, 65], BF16)
    nc.vector.tensor_copy(loP[:, 0:64], LH[:, 640 - 64 : 640])
    nc.vector.memset(loP[:, 64:65], -1.0)
    hiP = gpool.tile([128, 65], BF16)
    nc.vector.tensor_copy(hiP[:, 0:64], LH[:, 1280 - 64 : 1280])
    nc.vector.memset(hiP[:, 64:65], 1.0)
    idmb = gpool.tile([128, 128], BF16)
    nc.vector.tensor_copy(idmb[:], idm[:])
    lhsT1 = gpool.tile([65, 128], BF16)
    lhsT2 = gpool.tile([65, 128], BF16)
    with tc.tile_pool(name="ptd", bufs=2, space="PSUM") as ptd:
        loT = ptd.tile([65, 128], BF16, tag="dT")
        nc.tensor.transpose(loT[:], loP[:], idmb[:])
        nc.scalar.activation(lhsT1[:], loT[:], AF.Copy, scale=-1.0)
        hiT = ptd.tile([65, 128], BF16, tag="dT")
        nc.tensor.transpose(hiT[:], hiP[:], idmb[:])
        nc.scalar.activation(lhsT2[:], hiT[:], AF.Copy, scale=-1.0)
    sgn = gpool.tile([128, 2 * NPIX], BF16)

    # ---- raster: per face-tile one combined is_ge + 16 accum matmuls ----
    # Junk "warmer" matmuls keep the PE p-state ramped: a pre-raster burst
    # while the first compare runs, plus a couple per face-tile to bridge the
    # compare/accumulate rate gap without the engine ever going idle.
    LHv = LH[:].rearrange("p (s ft i) -> p s ft i", s=2, ft=NTILE)
    spool = ctx.enter_context(tc.tile_pool(name="ghp", bufs=3))
    pscnt = ctx.enter_context(tc.tile_pool(name="pcnt", bufs=1, space="PSUM"))
    pwarm = ctx.enter_context(tc.tile_pool(name="pwarm", bufs=1, space="PSUM"))
    cnt8 = pscnt.tile([8, 512], F32, tag="cnt8")
    wps = pwarm.tile([128, 480], F32, tag="wps")

    def warm(n):
        for wq in range(n):
            nc.tensor.matmul(wps[:], uvnhB[:], tb[:, 0:480], start=True,
                             stop=True)

    warm(2)
    pdif = ctx.enter_context(tc.tile_pool(name="pdif", bufs=1, space="PSUM"))
    nmm = 0
    NMM = (NTILE - 1) * 16

    def diff_half(h):
        side, hh = h // 2, h % 2
        lhsT = lhsT1 if side == 0 else lhsT2
        dp = pdif.tile([128, 2048], F32, tag="dp")
        for q in range(4):
            off = 2048 * hh + 512 * q
            nc.tensor.matmul(dp[:, 512 * q : 512 * (q + 1)], lhsT[:],
                             xb65[:, off : off + 512], start=True, stop=True)
        return dp

    def sign_half(h, dp):
        nc.scalar.activation(sgn[:, 2048 * h : 2048 * (h + 1)], dp[:], AF.Sign)

    def sgn_accum(slot):
        for c in range(4 * slot, 4 * slot + 4):
            q = c % 8
            nc.tensor.matmul(cnt8[:, :], oh1[:, 8 - q : 16 - q],
                             sgn[:, 512 * c : 512 * (c + 1)],
                             start=False, stop=False)

    dps = {0: diff_half(0)}
    prev_ghp = None
    for ft in range(NTILE - 2):
        ghp = spool.tile([128, 2 * NPIX], BF16, tag="ghp")
        lhb = LHv[:, :, ft, :].unsqueeze(2).broadcast_to([128, 2, IMG, IMG])
        nc.vector.tensor_tensor(
            ghp[:].rearrange("p (s j i) -> p s j i", s=2, j=IMG), xxv, lhb,
            OP.is_ge)
        for c in range(16):
            accum_chunk(c, ghp, 0, oh)
        if ft <= 3:
            sign_half(ft, dps.pop(ft))
            if ft < 3:
                dps[ft + 1] = diff_half(ft + 1)
        elif ft <= 7:
            sgn_accum(ft - 4)
        prev_ghp = ghp

    # last DVE face-tile as two pinned half-compares: the junk-copy pins
    # stop the scheduler from hoisting them, and the first 8 accumulations
    # overlap the second half's compare
    ftL = NTILE - 2
    lhbL = LHv[:, :, ftL, :].unsqueeze(2).broadcast_to([128, 2, IMG, IMG])
    gA = spool.tile([128, NPIX], BF16, tag="gA")
    nc.vector.tensor_copy(gA[:, 0:1], prev_ghp[:, 0:1])
    nc.vector.tensor_tensor(gA[:].rearrange("p (j i) -> p j i", j=IMG),
                            xxv[:, 0], lhbL[:, 0], OP.is_ge)
    gB = spool.tile([128, NPIX], BF16, tag="gB")
    nc.vector.tensor_copy(gB[:, 0:1], gA[:, 0:1])
    nc.vector.tensor_tensor(gB[:].rearrange("p (j i) -> p j i", j=IMG),
                            xxv[:, 1], lhbL[:, 1], OP.is_ge)
    for c in range(8):
        accum_chunk(c, gA, 0, oh)
    for c in range(8, 16):
        accum_chunk(c, gB, 8, oh)

    # ---- threshold: covered iff cnt >= NF + 1 ----
    silb = gpool.tile([8, 512], F32)
    nc.vector.tensor_scalar(silb[:], cnt8[:], 2.0 * (NF - 128) + 0.5, None,
                            OP.is_gt)
    nc.sync.dma_start(sil_d.ap(), silb[:])


_NC = None


def _get_program():
    global _NC
    if _NC is None:
        nc = bacc.Bacc(
            "TRN2",
            target_bir_lowering=False,
            debug=False,
            enable_asserts=False,
            num_devices=B,
        )
        from contextlib import ExitStack

        with tile.TileContext(nc) as tc:
            with ExitStack() as ctx:
                build_kernel(ctx, tc)
        nc.compile()
        _NC = nc
    return _NC


def _consts():
    """Input-independent constant tables (pixel grid, t-plane basis)."""
    j = np.arange(IMG, dtype=np.float32)
    xs = (2.0 * j - 63.0) / 64.0                      # exact in bf16
    ys = (63.0 - 2.0 * j) / 64.0
    xg = np.empty((2, IMG, IMG), dtype=np.float32)
    xg[0] = xs[:, None]
    xg[1] = -xs[:, None]
    xgrid = np.broadcast_to(xg.reshape(1, 2 * NPIX), (128, 2 * NPIX))
    xgrid = np.ascontiguousarray(xgrid).astype(ml_dtypes.bfloat16)
    tb = np.zeros((60, 1920), dtype=np.float32)
    for m in range(30):
        tb[2 * m, m * 64 : (m + 1) * 64] = ys
        tb[2 * m + 1, m * 64 : (m + 1) * 64] = 1.0
    tbasis = tb.astype(ml_dtypes.bfloat16)
    xb = np.zeros((65, NPIX), dtype=np.float32)
    for i in range(IMG):
        xb[i, i::IMG] = 1.0                    # onehot(i) over (j, i) columns
    xb[64] = np.repeat(xs, IMG)                # x_j
    xb65 = xb.astype(ml_dtypes.bfloat16)
    return xgrid, tbasis, xb65


def _host_layout(vertices, faces):
    """Pure indexing: gather per-face-corner vertices into [16, 1280] where
    row 4k+d / column ft*128+p holds coord d (d=3: 1.0) of corner k of face
    ft*128+p; corners are (a, b, c, a)."""
    faces4 = np.concatenate([faces, faces[:, :1]], axis=1)  # [1280, 4]
    out = []
    for b in range(B):
        vg = vertices[b][faces4]                      # [1280, 4, 3]
        vg4 = np.concatenate(
            [vg, np.ones((NF, 4, 1), dtype=np.float32)], axis=2)  # [1280,4,4]
        out.append(np.ascontiguousarray(
            vg4.transpose(1, 2, 0).reshape(16, NF).astype(np.float32)))
    return out


def kernel(vertices, viewpoints, faces, img_size):
    vertices = np.asarray(vertices, dtype=np.float32)
    viewpoints = np.asarray(viewpoints, dtype=np.float32)
    faces = np.asarray(faces, dtype=np.int32)
    assert int(img_size) == IMG and vertices.shape == (B, V, 3)

    nc = _get_program()
    vgts = _host_layout(vertices, faces)
    xgrid, tbasis, xb65 = _consts()
    in_maps = [
        {"vgt16": vgts[b], "eye": np.ascontiguousarray(viewpoints[b]),
         "xgrid": xgrid, "tbasis": tbasis, "xb65": xb65}
        for b in range(B)
    ]
    res = run_bass_kernel_spmd(nc, in_maps, core_ids=list(range(B)))
    # device pixel order is (j, i): transpose back to raster (i, j)
    sil = np.stack([
        res.results[b]["sil"].reshape(IMG, IMG).T for b in range(B)
    ])
    return sil.reshape(B, 1, IMG, IMG).astype(np.float32)


if __name__ == "__main__":
    rng = np.random.default_rng(0)
    verts = rng.standard_normal((B, V, 3), dtype=np.float32) * 0.5
    vps = rng.standard_normal((B, 3), dtype=np.float32)
    fcs = rng.integers(0, V, (NF, 3), dtype=np.int32)
    out = kernel(verts, vps, fcs, IMG)
    print(out.shape, out.sum())


# revision 53
# speedup vs baseline: 1.0620x; 1.0054x over previous
"""Trainium2 Bass kernel for nn_Mesh_Renderer: silhouette via scanline intervals.

Data-parallel over batch (core b renders view b). Host work is layout only
(gather vertices[faces], constant grid/basis tables, transpose the returned
image). All input-dependent math on device.

Device algorithm (per core):
  1. look_at camera basis from eye; projection folded as [w;1]^T @ [R^T; -R@eye]
     (40 K=4 f32 matmuls), perspective divide -> per-corner (xn, yn) [128, 40].
  2. Edge coefficients per (face, edge): e = A x + B y + C. For each pixel row
     y_i the face coverage in x is an interval [lo, hi]:
       t_k(i) = -(B_k y_i + C_k)/A_k ; edge k bounds from below iff
       sign(2*area)*A_k > 0. lo = max over lower-edges, -hi = max over upper
       (negated). Invisible/degenerate faces forced to a contributes-nothing
       interval via +-BIG offsets folded into the per-edge (u, v) small tiles;
       empty rows canonicalized with -hi' = min(-hi, -lo) (point interval).
     The t-planes t = u*y + v are evaluated by PE against a constant
     block-diagonal basis (tbasis), with (u, v) PE-transposed into lhsT.
  3. Raster: count(i,j) = sum_f([x_j >= lo] + [x_j <= hi]) = F + #covering.
     One DVE is_ge over [128, 8192] per 128-face tile computes both compares
     ([x | -x] vs [lo | -hi] broadcast over j); PE ones-matmuls accumulate
     over faces into PSUM cnt8 [8, 512] (sliding-onehot lhsT selects the row).
  4. silhouette = cnt >= F+1; DMA out; host transposes (j,i)->(i,j).
"""

import sys

if "/opt/trn_rl_repo" not in sys.path:
    sys.path.insert(0, "/opt/trn_rl_repo")

import ml_dtypes
import numpy as np

import concourse.bacc as bacc
import concourse.tile as tile
from concourse import mybir
from concourse.bass_utils import run_bass_kernel_spmd

F32 = mybir.dt.float32
BF16 = mybir.dt.bfloat16
I32 = mybir.dt.int32
OP = mybir.AluOpType
AF = mybir.ActivationFunctionType

B, V, NF, IMG = 8, 642, 1280, 64
NPIX = IMG * IMG          # 4096
NTILE = NF // 128         # 10 face tiles
NCOL = NF * 4             # 5120 gathered corners (a, b, c, a)
EPS = 1e-8
BIG = 1.0e30
TAN_T = float(np.tan(np.deg2rad(np.float32(15.0)).astype(np.float32)))


def _normalize3(nc, pool, v, name):
    """v [1,3] f32 -> v * rsqrt(sum v^2); margins cover the eps difference."""
    sq = pool.tile([1, 3], F32, name=f"{name}_sq")
    nc.vector.tensor_tensor(sq[:], v[:], v[:], OP.mult)
    s = pool.tile([1, 1], F32, name=f"{name}_s")
    nc.vector.tensor_reduce(s[:], sq[:], mybir.AxisListType.X, OP.add)
    n = pool.tile([1, 1], F32, name=f"{name}_n")
    nc.scalar.activation(n[:], s[:], AF.Sqrt)
    r = pool.tile([1, 1], F32, name=f"{name}_r")
    nc.vector.reciprocal(r[:], n[:])
    out = pool.tile([1, 3], F32, name=f"{name}_out")
    nc.vector.tensor_scalar(out[:], v[:], r[:], None, OP.mult)
    return out


def _cross3(nc, pool, a, b, name):
    a2 = pool.tile([1, 6], F32, name=f"{name}_a2")
    nc.vector.tensor_copy(a2[:, 0:3], a[:])
    nc.vector.tensor_copy(a2[:, 3:6], a[:])
    b2 = pool.tile([1, 6], F32, name=f"{name}_b2")
    nc.vector.tensor_copy(b2[:, 0:3], b[:])
    nc.vector.tensor_copy(b2[:, 3:6], b[:])
    m1 = pool.tile([1, 3], F32, name=f"{name}_m1")
    nc.vector.tensor_tensor(m1[:], a2[:, 1:4], b2[:, 2:5], OP.mult)
    m2 = pool.tile([1, 3], F32, name=f"{name}_m2")
    nc.vector.tensor_tensor(m2[:], a2[:, 2:5], b2[:, 1:4], OP.mult)
    out = pool.tile([1, 3], F32, name=f"{name}_out")
    nc.vector.tensor_tensor(out[:], m1[:], m2[:], OP.subtract)
    return out


def build_kernel(ctx, tc):
    nc = tc.nc
    vgt_d = nc.dram_tensor("vgt16", [16, NF], F32, kind="ExternalInput")
    eye_d = nc.dram_tensor("eye", [3], F32, kind="ExternalInput")
    xg_d = nc.dram_tensor("xgrid", [128, 2 * NPIX], BF16, kind="ExternalInput")
    tb_d = nc.dram_tensor("tbasis", [60, 1920], BF16, kind="ExternalInput")
    xb_d = nc.dram_tensor("xb65", [65, NPIX], BF16, kind="ExternalInput")
    sil_d = nc.dram_tensor("sil", [NPIX], F32, kind="ExternalOutput")

    cpool = ctx.enter_context(tc.tile_pool(name="cam", bufs=1))
    ppool = ctx.enter_context(tc.tile_pool(name="proj", bufs=1))
    gpool = ctx.enter_context(tc.tile_pool(name="grid", bufs=1))

    # ---- input DMAs ----
    eyeR = cpool.tile([1, 3], F32)
    nc.sync.dma_start(eyeR[:], eye_d.ap())
    vgt = gpool.tile([16, NF], F32)
    nc.sync.dma_start(vgt[:], vgt_d.ap())
    tb = gpool.tile([60, 1920], BF16)
    nc.sync.dma_start(tb[:], tb_d.ap())
    xb65 = gpool.tile([65, NPIX], BF16)
    nc.sync.dma_start(xb65[:], xb_d.ap())
    xx = gpool.tile([128, 2 * NPIX], BF16)
    xxv = xx[:].rearrange("p (s j i) -> p s j i", s=2, j=IMG)

    # identity for PE transposes (iotas on Pool, rest tiny)
    iop = gpool.tile([128, 1], I32)
    nc.gpsimd.iota(iop[:], pattern=[[1, 1]], base=0, channel_multiplier=1)
    iopf = gpool.tile([128, 1], F32)
    nc.vector.tensor_copy(iopf[:], iop[:])
    iof = gpool.tile([128, 128], I32)
    nc.gpsimd.iota(iof[:], pattern=[[1, 128]], base=0, channel_multiplier=0)
    ioff = gpool.tile([128, 128], F32)
    nc.vector.tensor_copy(ioff[:], iof[:])
    idm = gpool.tile([128, 128], F32)
    nc.vector.tensor_scalar(idm[:], ioff[:], iopf[:], None, OP.is_equal)

    # pixel-row y values for the DVE-side TLO build
    it32 = gpool.tile([128, IMG], I32)
    nc.gpsimd.iota(it32[:], pattern=[[1, IMG]], base=0, channel_multiplier=0)
    itf = gpool.tile([128, IMG], F32)
    nc.vector.tensor_copy(itf[:], it32[:])
    ysb = gpool.tile([128, IMG], BF16)   # y_i = (63 - 2i)/64, exact bf16
    nc.vector.tensor_scalar(ysb[:], itf[:], -1.0 / 32.0, 63.0 / 64.0, OP.mult,
                            OP.add)

    # sliding one-hot for row-targeted PE accumulation
    oh = gpool.tile([128, 16], BF16)
    nc.gpsimd.memset(oh[:], 0.0)
    nc.gpsimd.memset(oh[:, 8:9], 2.0)
    oh1 = gpool.tile([128, 16], BF16)
    nc.gpsimd.memset(oh1[:], 0.0)
    nc.gpsimd.memset(oh1[:, 8:9], 1.0)

    # ---- camera basis (partition 0, tiny tiles) ----
    # x_ax dir = cross(up, z) = cross(up, -eye) up to positive scale; the
    # x-chain (DVE) and z-chain (Pool/ACT) run in parallel. Normalized axes
    # are produced directly in duplicated [1, 6] form for the cross product.
    nege = cpool.tile([1, 3], F32)
    nc.vector.tensor_scalar(nege[:], eyeR[:], -1.0, None, OP.mult)
    xr = cpool.tile([1, 3], F32)
    nc.vector.memset(xr[:], 0.0)
    nc.vector.tensor_copy(xr[:, 0:1], nege[:, 2:3])
    nc.vector.tensor_scalar(xr[:, 2:3], nege[:, 0:1], -1.0, None, OP.mult)
    # x chain (DVE)
    sqx = cpool.tile([1, 3], F32)
    nc.vector.tensor_tensor(sqx[:], xr[:], xr[:], OP.mult)
    ssx = cpool.tile([1, 1], F32)
    nc.vector.tensor_reduce(ssx[:], sqx[:], mybir.AxisListType.X, OP.add)
    nx_ = cpool.tile([1, 1], F32)
    nc.scalar.activation(nx_[:], ssx[:], AF.Sqrt)
    rx_ = cpool.tile([1, 1], F32)
    nc.vector.reciprocal(rx_[:], nx_[:])
    xdup = cpool.tile([1, 6], F32)
    nc.vector.tensor_scalar(
        xdup[:].rearrange("p (two d) -> p two d", d=3),
        xr[:].unsqueeze(1).broadcast_to([1, 2, 3]), rx_[:], None, OP.mult)
    # z chain (Pool + ACT, parallel to the x chain)
    sqz = cpool.tile([1, 3], F32)
    nc.gpsimd.tensor_tensor(sqz[:], nege[:], nege[:], OP.mult)
    ssz1 = cpool.tile([1, 1], F32)
    nc.gpsimd.tensor_tensor(ssz1[:], sqz[:, 0:1], sqz[:, 1:2], OP.add)
    ssz = cpool.tile([1, 1], F32)
    nc.gpsimd.tensor_tensor(ssz[:], ssz1[:], sqz[:, 2:3], OP.add)
    nz_ = cpool.tile([1, 1], F32)
    nc.scalar.activation(nz_[:], ssz[:], AF.Sqrt)
    rz_ = cpool.tile([1, 1], F32)
    nc.vector.reciprocal(rz_[:], nz_[:])
    zdup = cpool.tile([1, 6], F32)
    nc.vector.tensor_scalar(
        zdup[:].rearrange("p (two d) -> p two d", d=3),
        nege[:].unsqueeze(1).broadcast_to([1, 2, 3]), rz_[:], None, OP.mult)
    # y = cross(z, x) of unit orthogonal vectors (unit up to ~1e-7)
    m1 = cpool.tile([1, 3], F32)
    nc.vector.tensor_tensor(m1[:], zdup[:, 1:4], xdup[:, 2:5], OP.mult)
    m2 = cpool.tile([1, 3], F32)
    nc.vector.tensor_tensor(m2[:], zdup[:, 2:5], xdup[:, 1:4], OP.mult)
    y_ax = cpool.tile([1, 3], F32)
    nc.vector.tensor_tensor(y_ax[:], m1[:], m2[:], OP.subtract)
    x_ax, z_ax = xdup, zdup

    # rt16 = 4 diagonal copies of rt4 = [R^T; -(eye^T @ R^T)] (one per corner),
    # staged row-major on partition 0 and reshaped by a single DMA.
    # stage[0, r*12 + c]; block k: rows 4k+d' cols 3k+d hold R[d, d'] and row
    # 4k+3 holds -Reye[d].
    rtT9 = cpool.tile([1, 9], F32)   # rtT9[0, 3*d' + d] = axis_d[d']
    for d, axis in enumerate([x_ax, y_ax, z_ax]):
        nc.vector.tensor_copy(
            rtT9[:].rearrange("p (dp d) -> p dp d", d=3)[:, :, d],
            axis[:, 0:3])
    # -Reye[d] = -sum_dp eye[dp] * R^T[dp, d] via elementwise + X-reduce
    el = cpool.tile([1, 9], F32)   # (d, dp) layout
    nc.vector.tensor_tensor(
        el[:].rearrange("p (d dp) -> p d dp", dp=3),
        rtT9[:].rearrange("p (dp d) -> p d dp", d=3),
        eyeR[:].unsqueeze(1).broadcast_to([1, 3, 3]), OP.mult)
    nreye0 = cpool.tile([1, 3], F32)
    nc.vector.tensor_reduce(nreye0[:], el[:].rearrange(
        "p (d dp) -> p d dp", dp=3), mybir.AxisListType.X, OP.add)
    nreye = cpool.tile([1, 3], F32)
    nc.vector.tensor_scalar(nreye[:], nreye0[:], -1.0, None, OP.mult)
    stage = cpool.tile([1, 192], F32)
    nc.vector.memset(stage[:], 0.0)
    rtv = rtT9[:].rearrange("p (dp d) -> p dp d", d=3)
    for k in range(4):
        base = 51 * k  # block k: coord rows at 51k + 12d' + d, ones at +36+d
        nc.vector.tensor_copy(
            stage[:, base : base + 36].rearrange(
                "p (dp c) -> p dp c", c=12)[:, :, 0:3], rtv)
        nc.vector.tensor_copy(stage[:, base + 36 : base + 39], nreye[:])
    rt16 = cpool.tile([16, 12], F32)
    nc.sync.dma_start(rt16[:], stage[:])

    # ---- projection: vca[p, (ft, k, d)] = [w;1]^T @ rt4 per corner ----
    vca = ppool.tile([128, 120], F32)
    with tc.tile_pool(name="pvc", bufs=1, space="PSUM") as psvc:
        vcp = psvc.tile([128, 120], F32)
        for ft in range(NTILE):
            nc.tensor.matmul(
                vcp[:, 12 * ft : 12 * (ft + 1)],
                vgt[:, 128 * ft : 128 * (ft + 1)],
                rt16[:],
                start=True,
                stop=True,
            )
        nc.vector.tensor_copy(vca[:], vcp[:])

    # junk write that depends on vca: pins the xgrid DMA behind the
    # projection in the scheduler so its long transfer cannot delay rt16's
    nc.vector.tensor_copy(xx[:, 0:1], vca[:, 0:1])
    nc.sync.dma_start(xx[:], xg_d.ap())
    # keep PE p-state ramped between projection and the T matmuls
    with tc.tile_pool(name="pwarm0", bufs=1, space="PSUM") as pwarm0:
        wps0 = pwarm0.tile([128, 480], F32, tag="wps0")
        for _ in range(14):
            nc.tensor.matmul(wps0[:], tb[:, 0:128], tb[:, 0:480], start=True,
                             stop=True)
    vcav = vca[:].rearrange("p (c d) -> p c d", d=3)
    vx, vy, vz = vcav[:, :, 0], vcav[:, :, 1], vcav[:, :, 2]

    # perspective divide (raw reciprocal; interval margins tolerate ~3e-3)
    dn = ppool.tile([128, 40], F32)
    nc.vector.tensor_scalar(dn[:], vz, TAN_T, EPS, OP.mult, OP.add)
    rc = ppool.tile([128, 40], F32)
    nc.vector.reciprocal(rc[:], dn[:])
    xn = ppool.tile([128, 40], F32)
    nc.vector.tensor_tensor(xn[:], vx, rc[:], OP.mult)
    yn = ppool.tile([128, 40], F32)
    nc.vector.tensor_tensor(yn[:], vy, rc[:], OP.mult)

    # visibility: all corner z > 0 (on Pool)
    vz4 = vca[:].rearrange("p (ft k d) -> p ft k d", k=4, d=3)
    mz1 = ppool.tile([128, 10], F32)
    nc.vector.tensor_tensor(mz1[:], vz4[:, :, 0, 2], vz4[:, :, 1, 2], OP.min)
    mz = ppool.tile([128, 10], F32)
    nc.vector.tensor_tensor(mz[:], mz1[:], vz4[:, :, 2, 2], OP.min)
    vg = ppool.tile([128, 10], F32)
    nc.vector.tensor_scalar(vg[:], mz[:], 0.0, None, OP.is_gt)

    # ---- edge coefficients [128, 30] in (ft, k) layout ----
    xn4 = xn[:].rearrange("p (ft k) -> p ft k", k=4)
    yn4 = yn[:].rearrange("p (ft k) -> p ft k", k=4)
    xk, xk1 = xn4[:, :, 0:3], xn4[:, :, 1:4]
    yk, yk1 = yn4[:, :, 0:3], yn4[:, :, 1:4]

    def t30(name, eng=None):
        return ppool.tile([128, 30], F32, name=name, tag=name)

    A = t30("A")
    Av = A[:].rearrange("p (ft k) -> p ft k", k=3)
    nc.vector.tensor_tensor(Av, yk, yk1, OP.subtract)
    Bc = t30("Bc")
    Bv = Bc[:].rearrange("p (ft k) -> p ft k", k=3)
    nc.vector.tensor_tensor(Bv, xk1, xk, OP.subtract)
    p1 = t30("p1")
    nc.gpsimd.tensor_tensor(p1[:].rearrange("p (ft k) -> p ft k", k=3), xk,
                            yk1, OP.mult)
    p2 = t30("p2")
    nc.gpsimd.tensor_tensor(p2[:].rearrange("p (ft k) -> p ft k", k=3), yk,
                            xk1, OP.mult)
    C = t30("C")
    nc.gpsimd.tensor_tensor(C[:], p1[:], p2[:], OP.subtract)

    Cv = C[:].rearrange("p (ft k) -> p ft k", k=3)
    S1 = ppool.tile([128, 10], F32, name="S1")
    nc.gpsimd.tensor_tensor(S1[:], Cv[:, :, 0], Cv[:, :, 1], OP.add)
    S = ppool.tile([128, 10], F32, name="S")
    nc.gpsimd.tensor_tensor(S[:], S1[:], Cv[:, :, 2], OP.add)

    # masks (Pool side-chain)
    w = t30("w")
    nc.gpsimd.tensor_tensor(w[:].rearrange("p (ft k) -> p ft k", k=3), Av,
                            S[:].unsqueeze(2).broadcast_to([128, 10, 3]),
                            OP.mult)
    # reciprocal side (DVE)
    iseq = t30("iseq")
    nc.vector.tensor_scalar(iseq[:], A[:], 0.0, None, OP.is_equal)
    Asafe = t30("Asafe")
    nc.vector.tensor_tensor(Asafe[:], A[:], iseq[:], OP.add)
    r0 = t30("r0")
    nc.vector.reciprocal(r0[:], Asafe[:])
    nr = t30("nr")
    nc.vector.tensor_scalar(nr[:], r0[:], -1.0, None, OP.mult)
    u = t30("u")
    nc.vector.tensor_tensor(u[:], Bc[:], nr[:], OP.mult)
    v = t30("v")
    nc.vector.tensor_tensor(v[:], C[:], nr[:], OP.mult)
    mpos = t30("mpos")
    nc.vector.tensor_scalar(mpos[:], w[:], 0.0, None, OP.is_gt)
    mneg = t30("mneg")
    nc.vector.tensor_scalar(mneg[:], w[:], 0.0, None, OP.is_lt)
    offlo = t30("offlo")
    nc.vector.tensor_scalar(offlo[:], mpos[:], BIG, -BIG, OP.mult, OP.add)
    offnh = t30("offnh")
    nc.vector.tensor_scalar(offnh[:], mneg[:], BIG, -BIG, OP.mult, OP.add)
    mnegN = t30("mnegN")
    nc.vector.tensor_scalar(mnegN[:], mneg[:], -1.0, None, OP.mult)

    sne = ppool.tile([128, 10], F32, name="sne")
    nc.vector.tensor_scalar(sne[:], S[:], 0.0, None, OP.not_equal)
    visq = ppool.tile([128, 10], F32, name="visq")
    nc.gpsimd.tensor_tensor(visq[:], vg[:], sne[:], OP.mult)
    ivq = ppool.tile([128, 10], F32, name="ivq")
    nc.vector.tensor_scalar(ivq[:], visq[:], -2.0 * BIG, 2.0 * BIG, OP.mult,
                            OP.add)
    ivqN = ppool.tile([128, 10], F32, name="ivqN")
    nc.vector.tensor_scalar(ivqN[:], visq[:], 2.0 * BIG, -2.0 * BIG, OP.mult,
                            OP.add)


    # (u, v) lhsT staging: only the negated-hi side goes through PE
    uvnh = ppool.tile([128, 60], F32, name="uvnh")
    uvnhv = uvnh[:].rearrange("p (m two) -> p m two", two=2)

    # lower side (computed on DVE directly): ulo = u*mpos ;
    # vlo = v*mpos - BIG*(1-mpos) + ivq
    ulo = t30("ulo")
    nc.vector.tensor_tensor(ulo[:], u[:], mpos[:], OP.mult)
    vlo1 = t30("vlo1")
    nc.vector.tensor_tensor(vlo1[:], v[:], mpos[:], OP.mult)
    vlo2 = t30("vlo2")
    nc.vector.tensor_tensor(vlo2[:], vlo1[:], offlo[:], OP.add)
    vlo = t30("vlo")
    nc.vector.tensor_tensor(
        vlo[:].rearrange("p (ft k) -> p ft k", k=3),
        vlo2[:].rearrange("p (ft k) -> p ft k", k=3),
        ivq[:].unsqueeze(2).broadcast_to([128, 10, 3]), OP.add)
    ulob = ppool.tile([128, 30], BF16, name="ulob")
    nc.vector.tensor_copy(ulob[:], ulo[:])
    vlob = ppool.tile([128, 30], BF16, name="vlob")
    nc.vector.tensor_copy(vlob[:], vlo[:])

    # negated upper side: unh = -u*mneg ; vnh = -v*mneg - BIG*(1-mneg) - ivq
    nc.vector.tensor_tensor(uvnhv[:, :, 0], u[:], mnegN[:], OP.mult)
    vnh1 = t30("vnh1")
    nc.vector.tensor_tensor(vnh1[:], v[:], mnegN[:], OP.mult)
    vnh2 = t30("vnh2")
    nc.vector.tensor_tensor(vnh2[:], vnh1[:], offnh[:], OP.add)
    nc.vector.tensor_tensor(
        uvnhv[:, :, 1].rearrange("p (ft k) -> p ft k", k=3),
        vnh2[:].rearrange("p (ft k) -> p ft k", k=3),
        ivqN[:].unsqueeze(2).broadcast_to([128, 10, 3]), OP.add)

    # ---- T planes: TLO on DVE (bf16 broadcast ops) in parallel with the
    # negated-hi side through PE (transpose + basis matmul) + ACT drain ----
    TLOs = gpool.tile([128, 1920], BF16)
    TLOa = gpool.tile([128, 1920], BF16)
    TNHs = gpool.tile([128, 1920], BF16)
    ub3 = ulob[:].rearrange("p (ft k) -> p ft k", k=3)
    vb3 = vlob[:].rearrange("p (ft k) -> p ft k", k=3)
    ysbc = ysb[:].rearrange("p (a b i) -> p a b i", a=1, b=1).broadcast_to(
        [128, NTILE, 3, IMG])
    TLOav = TLOa[:].rearrange("p (ft k i) -> p ft k i", k=3, i=IMG)
    nc.vector.tensor_tensor(TLOav, ub3.unsqueeze(3).broadcast_to(
        [128, NTILE, 3, IMG]), ysbc, OP.mult)
    nc.vector.tensor_tensor(
        TLOs[:].rearrange("p (ft k i) -> p ft k i", k=3, i=IMG), TLOav,
        vb3.unsqueeze(3).broadcast_to([128, NTILE, 3, IMG]), OP.add)
    with tc.tile_pool(name="ptr", bufs=1, space="PSUM") as ptr:
        uvnhT = ptr.tile([60, 128], F32, tag="uvT")
        nc.tensor.transpose(uvnhT[:], uvnh[:], idm[:])
        uvnhB = gpool.tile([60, 128], BF16)
        nc.scalar.activation(uvnhB[:], uvnhT[:], AF.Copy)
    with tc.tile_pool(name="ptp", bufs=1, space="PSUM") as ptp:
        TNHp = ptp.tile([128, 1920], F32, tag="tp")
        for q in range(4):
            nc.tensor.matmul(TNHp[:, 480 * q : 480 * (q + 1)], uvnhB[:],
                             tb[:, 480 * q : 480 * (q + 1)], start=True,
                             stop=True)
        nc.scalar.activation(TNHs[:], TNHp[:], AF.Copy)

    # ---- chains -> LH [128, 1280]: cols (s, ft, i); s=0: lo, s=1: -hi ----
    TLOv = TLOs[:].rearrange("p (ft k i) -> p ft k i", k=3, i=IMG)
    TNHv = TNHs[:].rearrange("p (ft k i) -> p ft k i", k=3, i=IMG)
    LH = gpool.tile([128, 2 * 640], BF16)
    lo1 = gpool.tile([128, 640], BF16)
    nc.vector.tensor_tensor(lo1[:], TLOv[:, :, 0, :], TLOv[:, :, 1, :], OP.max)
    nc.vector.tensor_tensor(
        LH[:, 0:640].rearrange("p (ft i) -> p ft i", i=IMG),
        lo1[:].rearrange("p (ft i) -> p ft i", i=IMG), TLOv[:, :, 2, :],
        OP.max)
    nh1 = gpool.tile([128, 640], BF16)
    nc.vector.tensor_tensor(nh1[:], TNHv[:, :, 0, :], TNHv[:, :, 1, :], OP.max)
    nh2 = gpool.tile([128, 640], BF16)
    nc.vector.tensor_tensor(
        nh2[:].rearrange("p (ft i) -> p ft i", i=IMG),
        nh1[:].rearrange("p (ft i) -> p ft i", i=IMG), TNHv[:, :, 2, :],
        OP.max)
    # canonicalize empty rows: -hi' = min(-hi, -lo)
    nlo = gpool.tile([128, 640], BF16)
    nc.vector.tensor_scalar(nlo[:], LH[:, 0:640], -1.0, None, OP.mult)
    nc.vector.tensor_tensor(LH[:, 640:1280], nh2[:], nlo[:], OP.min)

    # ---- face-tile 9 goes through PE diff-planes + ACT Sign ----
    # d1 = x - lo, d2 = hi - x as K=65 matmuls vs the constant pixel basis
    # xb65 (rows 0..63 = onehot(i), row 64 = x_j). lhsT rows hold -lo / +hi
    # (PE-transposed from LH) with the x-coefficient in row 64.
    ACT_FT = NTILE - 1
    loP = gpool.tile([128, 65], BF16)
    nc.vector.tensor_copy(loP[:, 0:64], LH[:, 640 - 64 : 640])
    nc.vector.memset(loP[:, 64:65], -1.0)
    hiP = gpool.tile([128, 65], BF16)
    nc.vector.tensor_copy(hiP[:, 0:64], LH[:, 1280 - 64 : 1280])
    nc.vector.memset(hiP[:, 64:65], 1.0)
    idmb = gpool.tile([128, 128], BF16)
    nc.vector.tensor_copy(idmb[:], idm[:])
    lhsT1 = gpool.tile([65, 128], BF16)
    lhsT2 = gpool.tile([65, 128], BF16)
    with tc.tile_pool(name="ptd", bufs=2, space="PSUM") as ptd:
        loT = ptd.tile([65, 128], BF16, tag="dT")
        nc.tensor.transpose(loT[:], loP[:], idmb[:])
        nc.scalar.activation(lhsT1[:], loT[:], AF.Copy, scale=-1.0)
        hiT = ptd.tile([65, 128], BF16, tag="dT")
        nc.tensor.transpose(hiT[:], hiP[:], idmb[:])
        nc.scalar.activation(lhsT2[:], hiT[:], AF.Copy, scale=-1.0)
    sgn = gpool.tile([128, 2 * NPIX], BF16)

    # ---- raster: per face-tile one combined is_ge + 16 accum matmuls ----
    # Junk "warmer" matmuls keep the PE p-state ramped: a pre-raster burst
    # while the first compare runs, plus a couple per face-tile to bridge the
    # compare/accumulate rate gap without the engine ever going idle.
    LHv = LH[:].rearrange("p (s ft i) -> p s ft i", s=2, ft=NTILE)
    spool = ctx.enter_context(tc.tile_pool(name="ghp", bufs=3))
    pscnt = ctx.enter_context(tc.tile_pool(name="pcnt", bufs=1, space="PSUM"))
    pwarm = ctx.enter_context(tc.tile_pool(name="pwarm", bufs=1, space="PSUM"))
    cnt8 = pscnt.tile([8, 512], F32, tag="cnt8")
    wps = pwarm.tile([128, 480], F32, tag="wps")

    def warm(n):
        for wq in range(n):
            nc.tensor.matmul(wps[:], uvnhB[:], tb[:, 0:480], start=True,
                             stop=True)

    warm(2)
    pdif = ctx.enter_context(tc.tile_pool(name="pdif", bufs=1, space="PSUM"))
    nmm = 0
    NMM = (NTILE - 1) * 16

    def diff_half(h):
        side, hh = h // 2, h % 2
        lhsT = lhsT1 if side == 0 else lhsT2
        dp = pdif.tile([128, 2048], F32, tag="dp")
        for q in range(4):
            off = 2048 * hh + 512 * q
            nc.tensor.matmul(dp[:, 512 * q : 512 * (q + 1)], lhsT[:],
                             xb65[:, off : off + 512], start=True, stop=True)
        return dp

    def sign_half(h, dp):
        nc.scalar.activation(sgn[:, 2048 * h : 2048 * (h + 1)], dp[:], AF.Sign)

    def sgn_accum(slot):
        for c in range(4 * slot, 4 * slot + 4):
            q = c % 8
            nc.tensor.matmul(cnt8[:, :], oh1[:, 8 - q : 16 - q],
                             sgn[:, 512 * c : 512 * (c + 1)],
                             start=False, stop=False)

    dps = {0: diff_half(0)}
    prev_ghp = None
    for ft in range(NTILE - 2):
        if ft == 0:
            # first face-tile as two half-compares so PE's accumulation
            # stream starts one half-compare earlier
            lhb = LHv[:, :, 0, :].unsqueeze(2).broadcast_to(
                [128, 2, IMG, IMG])
            gA0 = spool.tile([128, NPIX], BF16, tag="gA0", bufs=1)
            nc.vector.tensor_tensor(
                gA0[:].rearrange("p (j i) -> p j i", j=IMG), xxv[:, 0],
                lhb[:, 0], OP.is_ge)
            gB0 = spool.tile([128, NPIX], BF16, tag="gB0", bufs=1)
            nc.vector.tensor_copy(gB0[:, 0:1], gA0[:, 0:1])
            nc.vector.tensor_tensor(
                gB0[:].rearrange("p (j i) -> p j i", j=IMG), xxv[:, 1],
                lhb[:, 1], OP.is_ge)
            for c in range(8):
                accum_chunk(c, gA0, 0, oh)
            for c in range(8, 16):
                accum_chunk(c, gB0, 8, oh)
            sign_half(0, dps.pop(0))
            dps[1] = diff_half(1)
            prev_ghp = gB0
            continue
        ghp = spool.tile([128, 2 * NPIX], BF16, tag="ghp")
        lhb = LHv[:, :, ft, :].unsqueeze(2).broadcast_to([128, 2, IMG, IMG])
        nc.vector.tensor_tensor(
            ghp[:].rearrange("p (s j i) -> p s j i", s=2, j=IMG), xxv, lhb,
            OP.is_ge)
        for c in range(16):
            accum_chunk(c, ghp, 0, oh)
        if ft <= 3:
            sign_half(ft, dps.pop(ft))
            if ft < 3:
                dps[ft + 1] = diff_half(ft + 1)
        elif ft <= 7:
            sgn_accum(ft - 4)
        prev_ghp = ghp

    # last DVE face-tile as two pinned half-compares: the junk-copy pins
    # stop the scheduler from hoisting them, and the first 8 accumulations
    # overlap the second half's compare
    ftL = NTILE - 2
    lhbL = LHv[:, :, ftL, :].unsqueeze(2).broadcast_to([128, 2, IMG, IMG])
    gA = spool.tile([128, NPIX], BF16, tag="gA", bufs=1)
    nc.vector.tensor_copy(gA[:, 0:1], prev_ghp[:, 0:1])
    nc.vector.tensor_tensor(gA[:].rearrange("p (j i) -> p j i", j=IMG),
                            xxv[:, 0], lhbL[:, 0], OP.is_ge)
    gB = spool.tile([128, NPIX], BF16, tag="gB", bufs=1)
    nc.vector.tensor_copy(gB[:, 0:1], gA[:, 0:1])
    nc.vector.tensor_tensor(gB[:].rearrange("p (j i) -> p j i", j=IMG),
                            xxv[:, 1], lhbL[:, 1], OP.is_ge)
    for c in range(8):
        accum_chunk(c, gA, 0, oh)
    for c in range(8, 16):
        accum_chunk(c, gB, 8, oh)

    # ---- threshold: covered iff cnt >= NF + 1 ----
    silb = gpool.tile([8, 512], F32)
    nc.vector.tensor_scalar(silb[:], cnt8[:], 2.0 * (NF - 128) + 0.5, None,
                            OP.is_gt)
    nc.sync.dma_start(sil_d.ap(), silb[:])


_NC = None


def _get_program():
    global _NC
    if _NC is None:
        nc = bacc.Bacc(
            "TRN2",
            target_bir_lowering=False,
            debug=False,
            enable_asserts=False,
            num_devices=B,
        )
        from contextlib import ExitStack

        with tile.TileContext(nc) as tc:
            with ExitStack() as ctx:
                build_kernel(ctx, tc)
        nc.compile()
        _NC = nc
    return _NC


def _consts():
    """Input-independent constant tables (pixel grid, t-plane basis)."""
    j = np.arange(IMG, dtype=np.float32)
    xs = (2.0 * j - 63.0) / 64.0                      # exact in bf16
    ys = (63.0 - 2.0 * j) / 64.0
    xg = np.empty((2, IMG, IMG), dtype=np.float32)
    xg[0] = xs[:, None]
    xg[1] = -xs[:, None]
    xgrid = np.broadcast_to(xg.reshape(1, 2 * NPIX), (128, 2 * NPIX))
    xgrid = np.ascontiguousarray(xgrid).astype(ml_dtypes.bfloat16)
    tb = np.zeros((60, 1920), dtype=np.float32)
    for m in range(30):
        tb[2 * m, m * 64 : (m + 1) * 64] = ys
        tb[2 * m + 1, m * 64 : (m + 1) * 64] = 1.0
    tbasis = tb.astype(ml_dtypes.bfloat16)
    xb = np.zeros((65, NPIX), dtype=np.float32)
    for i in range(IMG):
        xb[i, i::IMG] = 1.0                    # onehot(i) over (j, i) columns
    xb[64] = np.repeat(xs, IMG)                # x_j
    xb65 = xb.astype(ml_dtypes.bfloat16)
    return xgrid, tbasis, xb65


def _host_layout(vertices, faces):
    """Pure indexing: gather per-face-corner vertices into [16, 1280] where
    row 4k+d / column ft*128+p holds coord d (d=3: 1.0) of corner k of face
    ft*128+p; corners are (a, b, c, a)."""
    faces4 = np.concatenate([faces, faces[:, :1]], axis=1)  # [1280, 4]
    out = []
    for b in range(B):
        vg = vertices[b][faces4]                      # [1280, 4, 3]
        vg4 = np.concatenate(
            [vg, np.ones((NF, 4, 1), dtype=np.float32)], axis=2)  # [1280,4,4]
        out.append(np.ascontiguousarray(
            vg4.transpose(1, 2, 0).reshape(16, NF).astype(np.float32)))
    return out


def kernel(vertices, viewpoints, faces, img_size):
    vertices = np.asarray(vertices, dtype=np.float32)
    viewpoints = np.asarray(viewpoints, dtype=np.float32)
    faces = np.asarray(faces, dtype=np.int32)
    assert int(img_size) == IMG and vertices.shape == (B, V, 3)

    nc = _get_program()
    vgts = _host_layout(vertices, faces)
    xgrid, tbasis, xb65 = _consts()
    in_maps = [
        {"vgt16": vgts[b], "eye": np.ascontiguousarray(viewpoints[b]),
         "xgrid": xgrid, "tbasis": tbasis, "xb65": xb65}
        for b in range(B)
    ]
    res = run_bass_kernel_spmd(nc, in_maps, core_ids=list(range(B)))
    # device pixel order is (j, i): transpose back to raster (i, j)
    sil = np.stack([
        res.results[b]["sil"].reshape(IMG, IMG).T for b in range(B)
    ])
    return sil.reshape(B, 1, IMG, IMG).astype(np.float32)


if __name__ == "__main__":
    rng = np.random.default_rng(0)
    verts = rng.standard_normal((B, V, 3), dtype=np.float32) * 0.5
    vps = rng.standard_normal((B, 3), dtype=np.float32)
    fcs = rng.integers(0, V, (NF, 3), dtype=np.int32)
    out = kernel(verts, vps, fcs, IMG)
    print(out.shape, out.sum())
